# revision 59
# baseline (speedup 1.0000x reference)
"""Trainium2 Bass kernel for nn_LossFunction_103079215159 (triplet-style loss
with online hard-negative mining).

Math (B=8192, D=256; x[:,0]=anchors, x[:,1]=positives):
  a = l2norm(x0), p = l2norm(x1)
  dist[i,j] = ||a_i - p_j + eps||, self-match excluded
  top5 smallest per row -> pick rank[i]-th (RNG-derived, data-independent)
  loss = mean relu(||a_i-p_i+eps||^2 - ||a_i-p_neg+eps||^2)

Reduction: with s[i,j] = <32*a_i, 16*p_j> = 512*cos_ij, smaller distance ==
larger s. Device mines the top-8 LARGEST s per row (diag included; dropped on
host by value-match against the exactly-known s_ii).

Device program (per core, 8-way data parallel over anchor rows):
  - inputs are host-prepped fp8e4 operands, pre-normalized, pre-scaled and
    pre-transposed into the PE's [contraction-partition, 2 k-tiles, cols]
    layout, so the device does NO norms / scaling / transposes.
  - PE: fp8 DoubleRow matmuls (contraction 256 in one instruction, 0.5
    cycles/out-col, 512 out-cols each) fill [128, 2048] f32 PSUM granules;
    4 granules per 128-row block, 2 PSUM slots in flight.
  - PSUM egress is the bottleneck (only ACT and DVE can read PSUM; Pool
    cannot touch it at all, and TensorTensor may read at most one PSUM
    operand):
      granules 1,2,3 -> ACT copies to SBUF bf16 (slot order a,b,a)
      granule 0      -> DVE Max8 straight from PSUM -> 8 exact candidates
  - DVE folds the three bf16 slabs (tensor_max at the 2x DVE rate) down to
    256 cols (bucket 16), then Max8 -> 8 tree candidates. 16 candidates
    per (row, block) stream out via per-block DMAs.
Host: normalize, fp8 prep, exact s_ii / pos_d2, diag drop-by-value, rank
select, relu + mean. All O(B*D) numpy, vs the device's O(B^2*D/8)/core.
"""

import base64

import ml_dtypes
import numpy as np

B = 8192
D = 256
NCORES = 8
M = B // NCORES  # 1024 anchor rows per core
RB = M // 128  # 8 row blocks per core
GW = 2048  # psum granule width (4 banks)
NG = B // GW  # 4 granules per row block
MMW = 512  # out cols per DoubleRow matmul
NACT = 3  # granules per row block evacuated by ACT (rest mined by DVE Max8)

ASCALE = 32.0  # anchors uploaded as 32*ahat  (32*16 = 512 = 256*s_scale)
PSCALE = 16.0  # positives uploaded as 16*phat
SS = ASCALE * PSCALE  # psum value = SS * <ahat_i, phat_j>

EPS = 1e-6
HARD_RANK = 5

# rank[i] in {0..4}: which of the 5 nearest negatives to use per row.
# Reproduces exactly:
#   k1, k2 = jax.random.split(jax.random.key(1))
#   coin = jax.random.uniform(k1, (8192,)) < 0.5
#   rank = jnp.where(coin, 0, jax.random.randint(k2, (8192,), 0, 5))
_RANK_B64 = (
    "AAIEAAAAAAAAAAIAAwAAAAAAAAAAAAMAAAIAAAMABAAAAAAAAwACAAABAAQCBAADAAACAgAEAwAC"
    "AAMEAAAAAwEEAQMAAAIAAgAAAAAAAAAEAAQAAwAABAECAAIAAAAAAgADAAACAwQABAAAAgMAAgAE"
    "AwAAAgACAAECAAEAAAECAQEBAAAABAACBAAAAAAAAAEAAAAEAQAAAAIAAgADAAEAAAAAAQAAAQME"
    "AgAAAAEEAAAAAAMAAQAAAAAEAAAEAQAAAAAAAAAAAAAAAAADAQQAAAAAAgABAAAAAAADAAADAAQA"
    "AAAAAwMAAAAEAAAAAAAAAAEAAAMAAAAAAAQAAAACAgAEAQAAAAABAAADAgABAAIAAAAAAwQCAAAD"
    "AgAAAAADAgAAAQAABAAABAAAAAAAAAIAAAEABAADAAAAAAAEAAAAAQEBAAAAAAMAAAIAAAAAAAMA"
    "AwIDAAEAAQQAAAIAAAEEAAECAAIAAAEAAAADAAIAAQICAAABAgAAAQAAAAIAAAADAAEDBAAAAQEA"
    "AgAAAAAEBAAAAAEAAgECAAIEAAAABAAEAQIABAAAAAAAAAAAAAMBAQAAAAMCAgADAAIDAwQDBAAE"
    "AAAAAAAAAAEAAAEAAwMAAAAAAAAAAAABAAAAAAAAAAEAAAADAgMAAAMAAAAAAAMAAQAAAAAAAgAA"
    "BAAAAAMBAQABAAAAAAAAAAIAAwAAAgAEAwABAAAAAAAAAAAAAAIAAgABAgAEAAABAQIAAgIDAgAE"
    "AAAAAAAAAQAABAAEAAAAAAAAAQIAAgAAAAMAAQACAAAAAAADAAQAAQABBAAEAAMABAABAQADAQAA"
    "AgABAgAEAAIAAAAAAgAAAwAAAwAAAAAEAAAAAAEAAAAAAAIEAAAAAgAABAEAAgAAAAAAAAEAAAAC"
    "AAECBAADAAAAAQAAAAIAAAAAAgMAAAAAAQAAAAQAAAAAAAMEAwEAAgEAAAAAAAAABAADAQIDAAAA"
    "AAEAAwAAAgAAAAEAAgAAAAAAAgAAAAAABAAEAAACAAIAAAQAAgADAAEAAAQAAAACAAECAwIEAAAA"
    "BAQAAAQABAMAAAQAAwIAAQMAAAQAAAACAAAEAAAABAAAAAAAAAMBAAEAAAQDAAAAAAQDAAAAAAIA"
    "AAAEAwACAAQAAgACAAACAQQAAAQDAgQDAQAAAAAEAAADBAECBAAEAAEBAAAAAAEAAgAAAwAAAgAB"
    "AwAAAgAEBAAAAAIEAAAAAwACAAIBAAABAwQAAQAAAAQAAAAAAAIAAAEBAAIAAAAAAAEAAAAAAAEB"
    "AAAAAgACAAAAAAMAAwAAAAAABAMABAMAAQQBAAQCAAEDAAAAAAIAAAAEAAMDAAAEAAEAAQAAAAAA"
    "AAICBAABAQQEAAAAAAQAAQABAAEEAAACBAAAAAMAAAAABAAAAAEBAAICAAIAAAAAAAAEBAAAAAMC"
    "AAQDAAABAAQCAAEAAAAABAQEAAIBAAAAAgAEAAEAAAIEBAACAAIAAAAABAMDBAQAAAAAAAIAAgAA"
    "AAACAAABAwMDAAAAAAAAAAACAQAAAwAAAAAEAAAAAAMAAAAAAgMAAAICAAMAAAAEAAAAAAABAAAA"
    "AAABAAAAAAMAAAEEAAIDAAEBAAQAAAMCAAAAAAAEAAACAAMAAAACAwAAAwAEAAAAAAQAAwABAAAC"
    "AwAAAAEABAQBAAIAAAIAAwAEAAEAAAACAgAAAAEEAAQAAAADAAMDAAQDBAABBAACAwAAAAAEAAMA"
    "AgQABAIAAAAEAAQCAQMAAAIBAAIAAAQEAAACAAEAAAAAAAEAAAABAAEAAAAABAAAAAAABAADAAAA"
    "BAABBAABAAADAAAAAAAAAAAAAQAAAAAAAAMAAQAAAQACAAAAAAACAAMAAAMAAwIBAAAABAAAAAMA"
    "AAAAAAABAAABAQIBAAAAAgAAAAAEAAAAAAQAAAAAAwAAAAAAAgAAAAAAAAAAAAACAgAAAAABBAAA"
    "AwACAAEDAAAAAAQAAQACAAAEAAAAAgAAAAIAAAMBAAAAAAIEAwAAAAQAAAMAAAMAAAAAAAAAAAMC"
    "BAQAAAMAAAEBAQAAAAAAAAIAAAMAAAMAAAAAAAIABAAAAAABAgAAAAAEAAQCAAIAAAIDAAMBAAAA"
    "AwAAAQADAwABAAADAAAEAwAAAAAABAMAAAEAAAAAAAAAAAAAAAAAAAAAAAACAAAAAAICAgACAAMA"
    "AAACAwAAAAIAAQAAAAAEAQAAAgAEAAEAAwAEAAAAAAAAAAQAAwAAAwAAAAQEAgAAAAMEAAAAAAAB"
    "AwQAAgADAgEDAAQDAAAAAAIAAAAAAAAAAAAABAQAAAEEBAABAAAAAQQAAAAABAAAAAMCAAAAAAAD"
    "BAAAAAEEAwIAAAADAAAAAAAEAAIAAAMBAAADAAAAAAAAAgAAAAMCAAAEAgACAAADAAAAAwABBAAD"
    "AAIAAAAAAQAABAADAAAAAAQAAQABAAMAAwADAAAAAAAAAAMEAwADAwQBAAAAAAMAAAAAAAEDAAAE"
    "AQAAAAAAAgAAAQAAAAICAAIEAAABBAACAAABAgAAAQAABAIDAgAEAAMAAAAAAAEEAAMDBAADBAAA"
    "BAAAAAADAAABAwADAAAAAAMAAAQAAQIAAAAAAwICAAIAAAIAAAAAAQAAAAICAAMAAAEAAgQAAAAA"
    "AAQAAAAABAAAAAEAAAIAAAAAAAAAAAAAAAMABAAAAAADAgAAAAAABAAABAAAAwICAAIAAAACBAAD"
    "AAAAAAADAAABAAAAAQAAAAACAgAEAAAAAAAEBAAAAAAAAAIABAQBAAAAAAAEAQAAAAIAAQADAAAD"
    "BAADAAAEBAQAAAACAAAEAAAEAAAEAAIBAAAAAgECAAAAAAMCAAIEAgADAAMAAAADAAEAAQAAAAAB"
    "BAADAQAAAAAAAQADAAAEBAIAAAIAAQIDAAACAwAAAAMAAAAAAAAAAAQABAMAAAIDAAABAgEAAAAB"
    "AAEBAAIEAwAABAACAAQAAwEAAAAAAAAAAAABAQAAAAMBBAMAAwQABAMABAAAAwMDAQQEAAABAAEB"
    "BAAAAAAAAAABAAEDAQQAAAAABAICAAIEAAMAAAAAAwADAAQDAAECAQAAAAAAAAAAAAMCAgAAAAIA"
    "AAQEAAAAAAEAAAAAAgEAAQQAAAAEBAQDBAICAAADAgIAAQAAAQABAgQCAAABAwAAAwABAAQDAAAA"
    "AAAEAAAAAgABAAAABAAABAAAAAAAAwAEAAAAAAMAAwAAAAAAAAABAAAAAwMAAQMAAAAAAgABAAAA"
    "AAMAAQAAAQACBAAAAQAAAAECAgMAAAAAAAMAAAAEAgAAAwQCAAIAAAIAAAAAAAADBAAAAQAAAAAA"
    "AAEEAAAAAAAAAgQAAAADAAADAAAAAAAAAAAAAAIBAAEEBAAAAAAEAAAAAwABAAIBAwAAAAMEAAAA"
    "AgIDBAMAAAABAAEAAAMBAAMCAAAAAAADAAIBAAADAAAAAAABAQAAAAIAAAAEAAEAAAAAAAAABAAE"
    "AAAAAAMAAgEAAQMAAAAAAAACAAMBAgABAwAAAAAEBAAAAQADAAEAAAMBAAAAAQIAAwABAgECAQMA"
    "AAAAAAACAAAAAAEAAAAAAAAEAAAAAAMEAwABAAAEAAAAAAAAAAECAQEAAAAAAAAAAAACAAAAAQAE"
    "AAQAAAACAAQAAAAAAAAAAAEAAAABAAQBAwIAAAAAAAQCAAEBAAIAAgAAAAMEAAAEAAACAQEAAAAA"
    "AAAAAAQAAQQCAAQEAgMDAAQAAAMAAAADAAAEAAEAAwAEBAQDAAACAAEAAAAABAMDAAMAAAEAAAQA"
    "AgMAAwAABAABAAIDAAQAAAICAAIAAAAAAAIEAgAAAgAEAwIAAAABAAAEAQAAAwAAAAACBAECAQAA"
    "AwAAAwQAAwQDAAAAAAACAQQDAAAAAAAEAAAAAwMBAAAAAAQAAAAAAgIAAAADBAADBAAEAAQABAAA"
    "BAAAAwQBAAAAAAACAAACAAIAAAAEAAEABAAAAgAAAAAAAAAAAAEEAAAAAwAAAQIAAAMAAQACAwQE"
    "AQABAwAAAAAAAAAAAAMBAAAABAIAAAAAAAIEAAAAAgAAAwAEAwADAAACAAEDAwQEAwAAAAAAAAAD"
    "AwACAAIDBAAABAAEAAAAAAACAgACAgICAAAAAAAAAAADAAIDAAQBAAMAAgAAAgAAAAAAAAAAAQAE"
    "AwQAAQAAAAIBAgAAAAEAAAQAAAAAAAIAAAABAQAAAwABBAADAwABAAIAAAAAAQQBAgIABAAAAAQC"
    "AAACAgMCAwQDAAAAAAACAAABAAICAAAAAgIAAAAAAQIAAAAAAAABAAAAAAAAAAAAAAIBBAQEAAQA"
    "AgQBAAEAAAAAAAAEAwAAAAAABAAAAQABAAAAAgAAAAEAAAMBAgMAAQAAAQAAAAQAAAQAAAAAAAAA"
    "AAEAAgIAAAIAAAAAAAAEAgAAAAIBAAAAAAAAAAIEAAAAAgIAAAQAAAAAAwAAAgIAAAIABAMAAQAA"
    "AAAAAAADAAAAAAAAAAADAQADBAAAAwAAAAAAAAABBAACAQAAAAABAgADAAAAAAAAAgADAAMAAAID"
    "AAIAAAAEAAAABAAAAAAAAwABAQECAwAAAAEAAAAAAAQAAAAAAAEEAAMAAAAEAAAAAAIAAwECAAAA"
    "AQAAAAABAAAAAAAABAAAAAQABAECAAIBAAECAAAAAAADAAACAgAEAAQAAAAAAAMABAAAAQEABAAA"
    "BAEAAwMEAAMAAAQABAQDBAAAAAAAAwAAAgEEAAABAAAAAAAAAAIDAgAEAQABAwACAAAEAQQEAAIA"
    "AAADAAABAgMEBAAAAAAAAgACAAAABAQAAAABAAAAAAMDAwEAAAAEAAMABAAEAwIAAAQAAQAEAAAA"
    "AgAAAAAAAAEAAAAAAAAAAwEAAAEAAgACAAAAAQADAAAAAAEAAAAAAAAABAECAAAAAAIAAAQBAgIA"
    "AwAAAAIAAAMAAAAEAAIAAAIAAQACAAAAAAAAAAAAAAMCAAADAAEBAgAAAwAAAwADAwADAAQAAAAA"
    "AAIBAwAAAQAAAAEAAAABAAAAAAAEAAEAAAQAAgQDAgEEAgMCBAAAAQIAAgAAAgIAAAABAAQAAAAA"
    "AAAAAAEAAAAAAwQAAAAAAwAEAAAAAAADAAAAAAAEAAABBAAAAAAAAwQEAAAAAgQAAAAEAgAAAAAA"
    "AAEAAAECAAAABAIEAAAAAgAAAAECAgAAAAMDAgAAAAIBAAAEAAAAAAAAAAQAAAMAAAAAAwAAAQQA"
    "AAEDAQADAAMAAAAAAAAAAAEAAAIEAAICAQAAAAIAAAAAAAEBAAEAAAAAAAACAAMDAAEAAQAAAAAA"
    "AAADAAADAAAAAAEBAwMBAwEAAAIBAAQAAAAAAAADAAAAAAEAAAMAAAABAwMAAAAAAwAABAAAAAAA"
    "AwIAAAIDBAAEAAAAAwIAAgAAAAAAAAAAAAIAAAAAAwADAAMABAMAAgQAAwAAAwAAAAAEAgADAQAE"
    "AAQAAgAEAAAAAAADAAMAAAADAgACAQQAAAAEAAEABAAAAwEABAABAgAEBAABAwMEAAAAAQAEAgEE"
    "AAMBAAAAAAAAAAAEAAAAAAEAAAABAAAAAwAAAQIAAAMAAAAAAAAAAAAAAAACAAACBAACAAAAAAIA"
    "AAICAAEAAQAAAwMAAwEBAwAEAAMDAAQCAAIEAAABBAABBAEEAAECAQMEAAAAAAACAwADBAIBAwAB"
    "AAAAAwACAgMCAAMAAAAAAwMAAAQAAAQAAQAAAAAAAAMABAQAAwAAAAEAAgABAAAABAEAAAAAAAAC"
    "AQIAAAAAAAMAAwIAAQACAQMEAwQAAAAEAAMAAQAAAAADAQABAAQAAAABAQMBAAAEAQAAAAAAAAAE"
    "AAAAAAIEAAAEAAAAAAAEAwEAAAAAAAIAAgAAAwEAAAEAAgAAAAMAAAQEAwAAAAADAQABAwAAAAAB"
    "AwADBAAEAQAAAwAABAAABAAAAAAAAAABAAAAAAMCAAAAAgEAAAQDAQAAAAMDAAAEAAIABAAAAAAA"
    "AQMEAAAAAAAAAAAAAAEEBAAEAAQDAAAAAAAAAgAAAAMAAwAAAAEAAAAAAgAAAQAAAgAEAAADBAAA"
    "AwABAAAAAwADAAICAAIAAAICAgMEAgAAAAAAAQACAAQBBAAAAQEBAAAAAAIAAAAAAgACAAIAAAAA"
    "AQAABAIDAAAAAAAAAAAAAAAEAAAAAAABAQAAAAAEAAAAAwABAwAAAAIEAAAABAEAAgMCAwACAAAC"
    "AAADAAAAAwAAAAMAAwMAAgACAAAAAAEDBAQAAwIDAAAAAAQCAgADAAADAgAAAAAAAwAAAAMBAQEA"
    "AwEAAwABAAAAAAMCAAAAAAADAAAABAQDBAAABAEAAwAAAAQEAAAAAwAAAgIBBAACAAABAAQAAAAD"
    "AAQABAICAAAEAQMAAAACBAEAAAIAAAMEAAAABAADAAAAAAIAAAMAAQAAAAABAAIAAAACAwMDAAAA"
    "AgACAAIEAAAAAAEEAAEAAAMDAAQEBAEAAAAAAAAAAAEAAgAEAAQAAAAEAAMABAABAQMAAQADAAID"
    "AAAAAAMCAgEAAwQAAgIAAAAEAAEAAAAAAAAABAAAAAAAAAQAAAAEAAAABAAAAAAAAAAAAAAAAAAA"
    "AAAEAwMAAQMAAwQAAQABAwACAAMAAAAAAAADAQAEAgAAAgIBAAQBBAAAAAAAAAQAAQAEAgAEAAIC"
    "AAIEAAIAAgAAAAADAAAABAQAAAACBAEEAwIABAACAAAAAAMABAABAAAAAAMAAAQAAAABAAMAAAAA"
    "AgACAAMAAAAAAwAAAAIAAAAAAAAAAAMEAAQEAAIAAQAAAAQDBAAAAAQABAMAAQQAAQAAAAEEAAMD"
    "AQAABAADAAAAAAABAgAAAAAABAIAAAABAAAABAABAgECAwMAAAACAgEABAABAAAAAgEBAAAEBAAC"
    "AAAAAgEAAAMAAAACAAAAAgMAAAAAAAQBAAAAAAACAQMCAAABAAADAAADAwABAAIAAAADAAADAQAA"
    "AAAABAACAAAAAAIAAAAABAMDBAQAAAAAAAQBAAQAAAAAAAAAAQAAAAEEAAMABAEAAAAEAgAAAAMA"
    "AAAAAgMCAgIAAAAAAgAAAAAAAAMAAAAAAAEAAAAAAgMBAAMAAAAABAMEAAQAAAMAAwACBAAEAAAB"
    "AAAAAAACBAQABAAEAgQAAAAEAQMDAAMAAAIEAQAEBAADAQIABAEDAAAAAgQABAADAAAAAgACBAMB"
    "AAMDAAAAAAAAAAIDAAAAAAIABAADAAAAAQAAAAAAAAAEAQAAAgABAAMDBAIBAAAABAADAAMEAwQA"
    "AAQCAAEAAwMAAAQBAAACAAABAAEAAAQCBAMBAgAAAAAAAAAABAQCAwMABAAAAAAAAAAAAAAAAQME"
    "AAAAAQAABAACAAMCAwEBAAACAgAAAgEAAAADAAAEBAAAAAAAAAABAAABAwMAAAMCAwAEAwIAAAQA"
    "BAICAAEBAAIAAAACAgIBAAAAAgQCAgAAAQQAAAAAAAAAAAMEAAADAwQABAACBAQAAwQAAQEDAQAA"
    "BAAAAAAAAwAAAAACAAMAAgMEAwEAAAAAAAEDAAAAAAIBAAQAAAMAAAMABAAEAAEEAwMAAAABBAAE"
    "AAIEAwAAAAAAAAMAAgQAAAMAAAEAAQIAAAMDBAAABAAAAAMAAAAEAAAEAAMAAAAAAAAAAAMAAAAE"
    "AAABAwAAAQAAAAEEAAAAAAIAAQAEAAAAAAADAAMAAAQDAAAAAgQCAgEAAAIBAAAAAAADBAIAAAMA"
    "AAQAAQQAAAACAAAAAAMAAgAAAQMAAAAAAQADAAIAAAAAAgAABAAAAAQEBAAEAQQAAwABAAACAAAA"
    "AAAAAAAAAAADAAAEAAABAgADAAIAAgEDAAADAAAAAAADAwQAAAMBAAAAAAAAAAAAAgABAQADAQQA"
    "BAAAAwAAAAABAAAAAAIDAAAAAwAEAAAAAQAAAAAAAwAAAAIDAAAAAwADAAQAAAEAAAECAAIABAAA"
    "BAAABAACAAMAAQAAAAIAAgIAAgAAAAQAAQACAAACAAABAAEBAAIDAAIABAAAAwEAAgMAAAAAAAMA"
    "BAACBAAAAAAABAABBAAEAAAAAQQAAQAAAAAEAgAAAAAAAwADAAAAAAAAAAMAAAAAAAEAAAAABAEA"
    "AAAEAgIAAAIAAAAAAAAAAAAAAAEEAAADAAAAAAEAAwAAAAMEAgAAAAAAAAAAAAIEAAEAAQAABAAA"
    "BAEAAAQAAwAAAwABAAIDAwQEAAAAAwQAAAQABAMAAAECAgACAAIDAAAAAQIEAAQABAQDAAAAAAAA"
    "AAAAAAAAAwABAwAAAQADAwIAAAAAAQABAAAAAAEABAQBAwABAAADAgAEAAIAAAMABAEAAAEAAQAA"
    "BAMAAwQCAwMAAQMCAwQAAwAAAAEABAAAAAEAAgEAAAAAAAAAAAAAAAAAAgAEAQAAAAEAAAAEAwAA"
    "AQIABAMEAAABAAMAAgEEAAIAAAEEAAABAAABAQAAAAAAAgIAAAAAAAADAgABBAMEAgACBAACBAQA"
    "AgADAAACAgQAAwADAwAEBAQAAAEBAAAABAECAAAAAAAABAACAAAEBAAAAAADAAAEAAMAAAIBAAAA"
    "AAQAAQAABAAAAAACAAEDAwAEBAAAAAAAAAACAQAAAAAEAAIAAAADAAAAAAIAAwAAAAEEBAAAAgAD"
    "AAAAAgEAAAQAAAEAAAAAAAIEAAMAAwQABAACAAEBAAEAAAEABAAAAAICBAQAAQAAAgIEAAAAAAAA"
    "AAAAAAAABAIBAAAAAgIAAAACAQAAAAABAAAAAAQEAgAEAAABAAAAAAAAAAEAAAMCAwAEBAMDBAAA"
    "AAABAAABAAEBAAABAwAAAAABAAABAwMAAAABAAMEAAAAAgAAAAQAAAACAAMAAAAAAAAAAAQAAAQD"
    "AAAABAABAAIAAAIAAAAAAAICAwACAwABAAAAAAQAAwADAgAAAAAAAgEABAIAAAAAAAABBAAAAAIC"
    "AAQAAAQAAAEAAwMDAAAAAQAEBAAAAAEAAAEBAAAAAgAAAwIABAADAAAEAgAAAAAABAAAAAAAAAAC"
    "AAQAAgAEAwAAAAAEAAMEBAEAAQACAAAEAAAABAAAAAAAAAAEAQQAAAQEAAQAAgAAAQEAAQAAAAQE"
    "AAABAAAAAAQABAAEAQAABAACAwACBAQEAAAAAQEAAQABAAAAAAAAAAAAAQAAAQAAAAAEAAACAAAA"
    "BAACAAEAAAAAAAMAAAIAAAMEAQAAAAIBAAIBAAAABAECAAAAAAAAAAABAAMBAAAAAwQAAgAAAwAA"
    "AwAEAQQAAwAAAQQAAwQAAAABAAABAAAEAAQAAAACAAABAAAAAAAAAQIAAAABAAAAAAICAAACAAIA"
    "AAADAgMCAAABAAAAAwACAAMABAAAAAAAAAAAAAAAAAIAAAAAAAQBAAAAAAECAQMBAAAAAAACAAAD"
    "AAAAAAQCAAQBAAACAAAAAAMAAwIAAgMAAAABAwMDBAAABAAEAAAAAAEBAAQCAQAEAAQABAIAAAID"
    "AAEAAQAAAAACAAQAAAABAAADAQECAAAAAAQAAAMABAACAAAAAAQAAAAAAAAAAQEDAAABAwQDAwIA"
    "BAAAAQADAAAAAgAEAwAABAABAQAABAABAAQAAgAAAAAAAAQAAAMBAAACBAAEAAEEAAAABAAABAAA"
    "AAAABAMDAAEBAAAAAAAEAgMAAAAEAgADAAACAgAAAAMAAAQBAQAAAQAEAgAAAAMDAAAAAAABBAAA"
    "AAAAAwQBAAIAAAABAAIAAAIABAMAAAAEAwMAAAABAAAAAwECBAAABAAAAAACAAAAAAAAAAAEAQIB"
    "AAAABAMAAAQCAwEBAgAAAAQAAQAAAAABAAAAAAIAAwACAwECAQAAAgMCAwAEAAAEAQQAAAAAAwAA"
    "AAMAAAMAAAAABAAAAAAAAAMAAAMEAAAAAAAEAAAAAAAAAAQAAwECAAQAAAAAAgAAAAAAAAAAAAAA"
    "AAAEAAADAwAAAAMCAAIAAAAAAwAAAgADAAACAAADAAAAAAMBAAEBAAECAAADAAAEAQMDBAACAAAC"
    "AAABAAACAAQAAAAAAQAAAAAAAQABAwQAAAQCAAAAAwMAAQADAAMAAAMAAAIAAAAAAAAAAAEEAAAA"
    "AAMAAAMEAAACAAAAAAMAAwIAAQMAAgIAAAIAAQAAAAAABAMAAAAAAgEAAAABAQEBAAQAAgQDAAAA"
    "BAMAAAEAAAAAAgIAAwMAAAAABAIAAAADAAECAgIAAAEBAAMBAAQAAgAAAAIAAAIAAAAAAAQEAAAD"
    "AQEEAQIDAAACAAACAAIEAAECAAAAAgMCAwACAAABAwAAAwAAAAAABAAEAAQDAAAAAAABAQEBAAAE"
    "AAAAAwAAAgAAAAADAAECAQMAAAABAAACAAAAAAAAAwMAAAIAAAIAAAEBAAIEAAAEAAAAAAAAAAMA"
    "AQQAAAMEAAMAAwMAAQAAAAAAAAMEAAQCAAIDAAMDBAQAAAAEAAEAAAMCAQACAgAAAAEDAAQAAwAA"
    "AAAAAQQAAAICBAMAAAEAAAAAAAQDAAAAAQAAAQADAAADAAAAAAAAAQAABAAAAAAAAQADAgICAQIA"
    "AAIBAAEAAwAAAAAAAAADAwAAAAAABAIAAAAAAAAEAAMABAAAAAAAAAQAAwQABAAAAAAAAAAAAwED"
    "AAMAAAAAAAAABAMAAAAAAwEAAgABAAAAAQAAAAACAAAAAAAEAQABAAABAQAAAQAAAAMAAgABAAMA"
    "AAAABAAEAQAAAAMABAAAAAEAAQAAAwQDAAACAAQEAAACAAAEBAAAAAMBAAABAAACAAAAAAQAAAAB"
    "AAADAQIBAAADAAEAAQAAAgMBAAADAAIDAAQAAAAAAQEBAQAAAgMAAAACAAAEAwABAAAAAAAEAAAD"
    "AAEEAwEAAQAAAQACAAEAAAMAAQMAAgAAAAIAAAQAAAAAAAIDAAAAAAA="
)

_RANK_CACHE = None


def _get_rank() -> np.ndarray:
    """rank[i]: which of the 5 nearest negatives the reference picks per row.

    Must reproduce the reference's jax.random draws bit-exactly; compute on
    the CPU jax backend when available, else use the embedded constant
    (generated the same way).
    """
    global _RANK_CACHE
    if _RANK_CACHE is not None:
        return _RANK_CACHE
    try:
        import jax
        import jax.numpy as jnp

        cpu = jax.devices("cpu")[0]
        with jax.default_device(cpu):
            k1, k2 = jax.random.split(jax.random.key(1))
            coin = jax.random.uniform(k1, (B,)) < 0.5
            rank = jnp.where(coin, 0, jax.random.randint(k2, (B,), 0, HARD_RANK))
            r = np.asarray(jax.device_get(rank)).astype(np.uint8)
    except Exception:
        r = np.frombuffer(base64.b64decode(_RANK_B64), dtype=np.uint8)
    assert r.shape == (B,)
    _RANK_CACHE = r
    return r


_NC_CACHE = None


def _build_nc():
    import os as _os

    NBUF = int(_os.environ.get("K_NBUF", "4"))

    import concourse.mybir as mybir
    import concourse.tile as tile
    from concourse import bacc

    F32 = mybir.dt.float32
    BF16 = mybir.dt.bfloat16
    FP8 = mybir.dt.float8e4
    PM = mybir.MatmulPerfMode

    nc = bacc.Bacc()
    # at8[p, t*M + i] = 32*ahat.T[t*128+p, i]   (core's anchor slab)
    at8 = nc.dram_tensor("at8", [128, 2 * M], FP8, kind="ExternalInput").ap()
    # pt8[p, k*2*GW + t*GW + j] = 16*phat.T[t*128+p, k*GW+j]: column chunks of
    # GW cols, both k-tiles packed per chunk, so each chunk DMAs separately
    pt8 = nc.dram_tensor("pt8", [128, 2 * B], FP8, kind="ExternalInput").ap()
    # 8 direct candidates (f32) + 8 tree candidates (bf16) per (row, rb)
    cd = nc.dram_tensor("cd", [128, RB * 8], F32, kind="ExternalOutput").ap()
    ct = nc.dram_tensor("ct", [128, RB * 8], BF16, kind="ExternalOutput").ap()

    with tile.TileContext(nc) as tc:
        with (
            tc.tile_pool(name="ops", bufs=1) as opsp,
            tc.tile_pool(name="evac", bufs=NBUF) as evacp,
            tc.tile_pool(name="tree", bufs=NBUF) as treep,
            tc.tile_pool(name="out", bufs=2) as outp,
            tc.tile_pool(name="ps", bufs=2, space="PSUM") as psp,
        ):
            # at8 packed [p, rb*256 + t*128 + m]; rb0's slice lands first so
            # the pipeline starts as soon as column chunk 1 arrives
            # PE warmup fodder: ramp the p-state during the DMA head
            warm = opsp.tile([128, 128], FP8, tag="warm")
            nc.gpsimd.memset(warm, 0.0)
            # rb0's anchor slice is its own tile so its matmuls don't wait
            # for the full anchor DMA (deps are tile-granular)
            a0 = opsp.tile([128, 256], FP8, tag="a0")
            nc.sync.dma_start(a0, at8[:, :256])
            a_sb = opsp.tile([128, 2 * M - 256], FP8)
            # chunk 1 lands as two separate tiles (tile-granular DMA deps)
            # so granule 1's first matmuls start on the first half
            HW = GW // 2
            pc1a = opsp.tile([128, 2 * HW], FP8, tag="pc1a", name="pc1a")
            nc.sync.dma_start(pc1a, pt8[:, 2 * GW : 2 * GW + 2 * HW])
            pc1b = opsp.tile([128, 2 * HW], FP8, tag="pc1b", name="pc1b")
            nc.sync.dma_start(pc1b, pt8[:, 2 * GW + 2 * HW : 4 * GW])
            nc.sync.dma_start(a_sb, at8[:, 256:])
            pc1h = [
                pc1a.rearrange("p (t n) -> p t n", t=2),
                pc1b.rearrange("p (t n) -> p t n", t=2),
            ]
            pcol = [None, None, None, None]
            for k in (2, 3, 0):
                pc = opsp.tile([128, 2 * GW], FP8, tag=f"pc{k}", name=f"pc{k}")
                nc.sync.dma_start(pc, pt8[:, k * 2 * GW : (k + 1) * 2 * GW])
                pcol[k] = pc.rearrange("p (t n) -> p t n", t=2)
            a0v = a0.rearrange("p (t m) -> p t m", t=2)
            lhsT_all = a_sb.rearrange("p (r t m) -> p r t m", r=RB - 1, t=2)

            for rb in range(RB):
                lhsT = a0v if rb == 0 else lhsT_all[:, rb - 1]
                evs = []
                hs = []  # per-granule halved [128, GW//2] bf16

                def granule(gi, lhsT=lhsT, rb=rb):
                    ps = psp.tile([128, GW], F32, tag="ps")
                    if rb == 0 and gi == 1:
                        # warmup matmuls: ramp PE while DMAs land (the real
                        # matmuls below overwrite with start=True)
                        wv = warm.rearrange("p (t n) -> p t n", t=2)
                        for _ in range(2):
                            nc.tensor.matmul(
                                ps[:64, :64],
                                wv,
                                wv,
                                start=True,
                                stop=True,
                                perf_mode=PM.DoubleRow,
                            )
                    for q in range(GW // MMW):
                        if gi == 1:
                            rhs = pc1h[q // 2][
                                :, :, (q % 2) * MMW : (q % 2 + 1) * MMW
                            ]
                        else:
                            rhs = pcol[gi][:, :, q * MMW : (q + 1) * MMW]
                        nc.tensor.matmul(
                            ps[:, q * MMW : (q + 1) * MMW],
                            lhsT,
                            rhs,
                            start=True,
                            stop=True,
                            perf_mode=PM.DoubleRow,
                        )
                    return ps

                def direct(gi, rb=rb):
                    c8 = outp.tile([128, 8], F32, tag="c8")
                    nc.vector.max(out=c8, in_=granule(gi))
                    nc.sync.dma_start(cd[:, rb * 8 : (rb + 1) * 8], c8)

                def evac(gi, halve=True):
                    ev = evacp.tile([128, GW], BF16, tag=f"e{len(evs)}")
                    nc.scalar.copy(ev, granule(gi))
                    evs.append(ev)
                    if halve:
                        h = treep.tile([128, GW // 2], BF16, tag=f"h{len(hs)}")
                        nc.vector.tensor_max(
                            h, ev[:, : GW // 2], ev[:, GW // 2 :]
                        )
                        hs.append(h)

                # ACT granules land in slot order S1,S2,S1 (no in-block
                # bubble); the DVE-mined granule fills S2 after ACT's second
                # drain.  For the last block the DVE granule goes first so
                # its (late-slot) Max8 is off the tail's critical path.
                if rb < RB - 1:
                    evac(1)
                    evac(2)
                    evac(3, halve=False)
                    direct(0)
                else:
                    direct(0)
                    evac(1)
                    evac(2)
                    evac(3, halve=False)
                h = treep.tile([128, GW // 2], BF16, tag="h2")
                nc.vector.tensor_max(
                    h, evs[2][:, : GW // 2], evs[2][:, GW // 2 :]
                )
                hs.append(h)
                # merge chain (bucket 16)
                m1 = treep.tile([128, GW // 2], BF16, tag="m1")
                nc.vector.tensor_max(m1, hs[0], hs[1])
                m2 = treep.tile([128, GW // 2], BF16, tag="m2")
                nc.vector.tensor_max(m2, m1, hs[2])
                q1 = treep.tile([128, GW // 4], BF16, tag="q1")
                nc.vector.tensor_max(q1, m2[:, : GW // 4], m2[:, GW // 4 :])
                q2 = treep.tile([128, GW // 8], BF16, tag="q2")
                nc.vector.tensor_max(q2, q1[:, : GW // 8], q1[:, GW // 8 :])
                t8 = outp.tile([128, 8], BF16, tag="t8")
                nc.vector.max(out=t8, in_=q2)
                nc.sync.dma_start(ct[:, rb * 8 : (rb + 1) * 8], t8)

    nc.compile()
    return nc


def _get_nc():
    global _NC_CACHE
    if _NC_CACHE is None:
        _NC_CACHE = _build_nc()
    return _NC_CACHE


def _prep(x: np.ndarray):
    """Host prep: normalize, scale, transpose, interleave, fp8-quantize."""
    x = np.ascontiguousarray(np.asarray(x, dtype=np.float32))
    assert x.shape == (B, 2, D)
    x0 = x[:, 0, :]
    x1 = x[:, 1, :]
    na = np.sqrt(np.sum(x0 * x0, axis=1, keepdims=True))
    np_ = np.sqrt(np.sum(x1 * x1, axis=1, keepdims=True))
    ahat = x0 / np.maximum(na, 1e-12)
    phat = x1 / np.maximum(np_, 1e-12)

    a8 = (ASCALE * ahat).astype(ml_dtypes.float8_e4m3)
    p8 = (PSCALE * phat).astype(ml_dtypes.float8_e4m3)
    # the device sees the fp8-rounded values; use them for the exact diag
    a8f = a8.astype(np.float32)
    p8f = p8.astype(np.float32)
    sii_dev = np.einsum("ij,ij->i", a8f, p8f)  # approx of diag psum value

    aT = np.ascontiguousarray(a8.T)  # [D, B]
    pT = np.ascontiguousarray(p8.T)  # [D, B]

    pt8 = np.empty((128, 2 * B), dtype=ml_dtypes.float8_e4m3)
    for k in range(NG):
        if k == 1:
            # chunk 1 is stored as two half-chunks of GW//2 cols each
            hw = GW // 2
            for h in range(2):
                base = k * 2 * GW + h * 2 * hw
                c0 = k * GW + h * hw
                for t in range(2):
                    pt8[:, base + t * hw : base + (t + 1) * hw] = pT[
                        t * 128 : (t + 1) * 128, c0 : c0 + hw
                    ]
        else:
            for t in range(2):
                pt8[:, k * 2 * GW + t * GW : k * 2 * GW + (t + 1) * GW] = pT[
                    t * 128 : (t + 1) * 128, k * GW : (k + 1) * GW
                ]

    in_maps = []
    for c in range(NCORES):
        # at8[p, rb*256 + t*128 + m] = aT[t*128+p, c*M + rb*128 + m]
        at8 = np.empty((128, 2 * M), dtype=ml_dtypes.float8_e4m3)
        for rb in range(RB):
            for t in range(2):
                lo = rb * 256 + t * 128
                at8[:, lo : lo + 128] = aT[
                    t * 128 : (t + 1) * 128,
                    c * M + rb * 128 : c * M + (rb + 1) * 128,
                ]
        in_maps.append({"at8": np.ascontiguousarray(at8), "pt8": pt8})
    return in_maps, ahat, phat, sii_dev


def _epilogue(cands: np.ndarray, ahat, phat, sii_dev) -> np.float32:
    """cands: [B, 16] raw psum-scale candidate values, unsorted."""
    rank = _get_rank()

    order = np.argsort(-cands, axis=1)
    csort = np.take_along_axis(cands, order, axis=1)  # desc [B, 24]

    # drop the self-match: closest candidate to the (device-precision) diag
    # value, if within the fp8 noise band
    TOL = 8.0  # psum units; fp8 dot noise sigma ~1.7, bf16 evac ~0.5
    diff = np.abs(csort - sii_dev[:, None])
    kstar = np.argmin(diff, axis=1)
    hit = diff[np.arange(B), kstar] < TOL
    # shift left past the dropped slot where hit
    idx = np.arange(8)[None, :] + (
        hit[:, None] & (np.arange(8)[None, :] >= kstar[:, None])
    )
    top = np.take_along_axis(csort, idx, axis=1)  # [B, 8] diag-free

    s_sel = top[np.arange(B), rank] / SS  # = <ahat_i, phat_neg>
    ra = np.sum(ahat, axis=1)
    pos_d2 = np.sum(np.square(ahat - phat + EPS), axis=1)
    neg_d2 = 2.0 - 2.0 * s_sel + 2.0 * EPS * ra + D * EPS * EPS
    return np.float32(np.mean(np.maximum(pos_d2 - neg_d2, 0.0)))


def kernel(x: np.ndarray, _want_timing: bool = False):
    """x: [8192, 2, 256] float32 -> scalar float32 loss (0-d ndarray)."""
    from concourse.bass_utils import run_bass_kernel_spmd

    in_maps, ahat, phat, sii_dev = _prep(x)
    nc = _get_nc()
    res = run_bass_kernel_spmd(nc, in_maps, list(range(NCORES)))

    cands = np.empty((B, 16), dtype=np.float32)
    for c in range(NCORES):
        cdv = res.results[c]["cd"]  # [128, RB*8] f32
        ctv = res.results[c]["ct"].astype(np.float32)  # [128, RB*8]
        for rb in range(RB):
            rows = slice(c * M + rb * 128, c * M + (rb + 1) * 128)
            cands[rows, 0:8] = cdv[:, rb * 8 : (rb + 1) * 8]
            cands[rows, 8:16] = ctv[:, rb * 8 : (rb + 1) * 8]

    out = _epilogue(cands, ahat, phat, sii_dev)
    if _want_timing:
        return np.asarray(out), res, cands
    return np.asarray(out)


if __name__ == "__main__":
    rng = np.random.default_rng(0)
    x = rng.standard_normal((B, 2, D)).astype(np.float32)
    print(kernel(x))


# revision 60
# speedup vs baseline: 1.0061x; 1.0061x over previous
"""Trainium2 Bass kernel for nn_LossFunction_103079215159 (triplet-style loss
with online hard-negative mining).

Math (B=8192, D=256; x[:,0]=anchors, x[:,1]=positives):
  a = l2norm(x0), p = l2norm(x1)
  dist[i,j] = ||a_i - p_j + eps||, self-match excluded
  top5 smallest per row -> pick rank[i]-th (RNG-derived, data-independent)
  loss = mean relu(||a_i-p_i+eps||^2 - ||a_i-p_neg+eps||^2)

Reduction: with s[i,j] = <32*a_i, 16*p_j> = 512*cos_ij, smaller distance ==
larger s. Device mines the top-8 LARGEST s per row (diag included; dropped on
host by value-match against the exactly-known s_ii).

Device program (per core, 8-way data parallel over anchor rows):
  - inputs are host-prepped fp8e4 operands, pre-normalized, pre-scaled and
    pre-transposed into the PE's [contraction-partition, 2 k-tiles, cols]
    layout, so the device does NO norms / scaling / transposes.
  - PE: fp8 DoubleRow matmuls (contraction 256 in one instruction, 0.5
    cycles/out-col, 512 out-cols each) fill [128, 2048] f32 PSUM granules;
    4 granules per 128-row block, 2 PSUM slots in flight.
  - PSUM egress is the bottleneck (only ACT and DVE can read PSUM; Pool
    cannot touch it at all, and TensorTensor may read at most one PSUM
    operand):
      granules 1,2,3 -> ACT copies to SBUF bf16 (slot order a,b,a)
      granule 0      -> DVE Max8 straight from PSUM -> 8 exact candidates
  - DVE folds the three bf16 slabs (tensor_max at the 2x DVE rate) down to
    256 cols (bucket 16), then Max8 -> 8 tree candidates. 16 candidates
    per (row, block) stream out via per-block DMAs.
Host: normalize, fp8 prep, exact s_ii / pos_d2, diag drop-by-value, rank
select, relu + mean. All O(B*D) numpy, vs the device's O(B^2*D/8)/core.
"""

import base64

import ml_dtypes
import numpy as np

B = 8192
D = 256
NCORES = 8
M = B // NCORES  # 1024 anchor rows per core
RB = M // 128  # 8 row blocks per core
GW = 2048  # psum granule width (4 banks)
NG = B // GW  # 4 granules per row block
MMW = 512  # out cols per DoubleRow matmul
NACT = 3  # granules per row block evacuated by ACT (rest mined by DVE Max8)

ASCALE = 32.0  # anchors uploaded as 32*ahat  (32*16 = 512 = 256*s_scale)
PSCALE = 16.0  # positives uploaded as 16*phat
SS = ASCALE * PSCALE  # psum value = SS * <ahat_i, phat_j>

EPS = 1e-6
HARD_RANK = 5

# rank[i] in {0..4}: which of the 5 nearest negatives to use per row.
# Reproduces exactly:
#   k1, k2 = jax.random.split(jax.random.key(1))
#   coin = jax.random.uniform(k1, (8192,)) < 0.5
#   rank = jnp.where(coin, 0, jax.random.randint(k2, (8192,), 0, 5))
_RANK_B64 = (
    "AAIEAAAAAAAAAAIAAwAAAAAAAAAAAAMAAAIAAAMABAAAAAAAAwACAAABAAQCBAADAAACAgAEAwAC"
    "AAMEAAAAAwEEAQMAAAIAAgAAAAAAAAAEAAQAAwAABAECAAIAAAAAAgADAAACAwQABAAAAgMAAgAE"
    "AwAAAgACAAECAAEAAAECAQEBAAAABAACBAAAAAAAAAEAAAAEAQAAAAIAAgADAAEAAAAAAQAAAQME"
    "AgAAAAEEAAAAAAMAAQAAAAAEAAAEAQAAAAAAAAAAAAAAAAADAQQAAAAAAgABAAAAAAADAAADAAQA"
    "AAAAAwMAAAAEAAAAAAAAAAEAAAMAAAAAAAQAAAACAgAEAQAAAAABAAADAgABAAIAAAAAAwQCAAAD"
    "AgAAAAADAgAAAQAABAAABAAAAAAAAAIAAAEABAADAAAAAAAEAAAAAQEBAAAAAAMAAAIAAAAAAAMA"
    "AwIDAAEAAQQAAAIAAAEEAAECAAIAAAEAAAADAAIAAQICAAABAgAAAQAAAAIAAAADAAEDBAAAAQEA"
    "AgAAAAAEBAAAAAEAAgECAAIEAAAABAAEAQIABAAAAAAAAAAAAAMBAQAAAAMCAgADAAIDAwQDBAAE"
    "AAAAAAAAAAEAAAEAAwMAAAAAAAAAAAABAAAAAAAAAAEAAAADAgMAAAMAAAAAAAMAAQAAAAAAAgAA"
    "BAAAAAMBAQABAAAAAAAAAAIAAwAAAgAEAwABAAAAAAAAAAAAAAIAAgABAgAEAAABAQIAAgIDAgAE"
    "AAAAAAAAAQAABAAEAAAAAAAAAQIAAgAAAAMAAQACAAAAAAADAAQAAQABBAAEAAMABAABAQADAQAA"
    "AgABAgAEAAIAAAAAAgAAAwAAAwAAAAAEAAAAAAEAAAAAAAIEAAAAAgAABAEAAgAAAAAAAAEAAAAC"
    "AAECBAADAAAAAQAAAAIAAAAAAgMAAAAAAQAAAAQAAAAAAAMEAwEAAgEAAAAAAAAABAADAQIDAAAA"
    "AAEAAwAAAgAAAAEAAgAAAAAAAgAAAAAABAAEAAACAAIAAAQAAgADAAEAAAQAAAACAAECAwIEAAAA"
    "BAQAAAQABAMAAAQAAwIAAQMAAAQAAAACAAAEAAAABAAAAAAAAAMBAAEAAAQDAAAAAAQDAAAAAAIA"
    "AAAEAwACAAQAAgACAAACAQQAAAQDAgQDAQAAAAAEAAADBAECBAAEAAEBAAAAAAEAAgAAAwAAAgAB"
    "AwAAAgAEBAAAAAIEAAAAAwACAAIBAAABAwQAAQAAAAQAAAAAAAIAAAEBAAIAAAAAAAEAAAAAAAEB"
    "AAAAAgACAAAAAAMAAwAAAAAABAMABAMAAQQBAAQCAAEDAAAAAAIAAAAEAAMDAAAEAAEAAQAAAAAA"
    "AAICBAABAQQEAAAAAAQAAQABAAEEAAACBAAAAAMAAAAABAAAAAEBAAICAAIAAAAAAAAEBAAAAAMC"
    "AAQDAAABAAQCAAEAAAAABAQEAAIBAAAAAgAEAAEAAAIEBAACAAIAAAAABAMDBAQAAAAAAAIAAgAA"
    "AAACAAABAwMDAAAAAAAAAAACAQAAAwAAAAAEAAAAAAMAAAAAAgMAAAICAAMAAAAEAAAAAAABAAAA"
    "AAABAAAAAAMAAAEEAAIDAAEBAAQAAAMCAAAAAAAEAAACAAMAAAACAwAAAwAEAAAAAAQAAwABAAAC"
    "AwAAAAEABAQBAAIAAAIAAwAEAAEAAAACAgAAAAEEAAQAAAADAAMDAAQDBAABBAACAwAAAAAEAAMA"
    "AgQABAIAAAAEAAQCAQMAAAIBAAIAAAQEAAACAAEAAAAAAAEAAAABAAEAAAAABAAAAAAABAADAAAA"
    "BAABBAABAAADAAAAAAAAAAAAAQAAAAAAAAMAAQAAAQACAAAAAAACAAMAAAMAAwIBAAAABAAAAAMA"
    "AAAAAAABAAABAQIBAAAAAgAAAAAEAAAAAAQAAAAAAwAAAAAAAgAAAAAAAAAAAAACAgAAAAABBAAA"
    "AwACAAEDAAAAAAQAAQACAAAEAAAAAgAAAAIAAAMBAAAAAAIEAwAAAAQAAAMAAAMAAAAAAAAAAAMC"
    "BAQAAAMAAAEBAQAAAAAAAAIAAAMAAAMAAAAAAAIABAAAAAABAgAAAAAEAAQCAAIAAAIDAAMBAAAA"
    "AwAAAQADAwABAAADAAAEAwAAAAAABAMAAAEAAAAAAAAAAAAAAAAAAAAAAAACAAAAAAICAgACAAMA"
    "AAACAwAAAAIAAQAAAAAEAQAAAgAEAAEAAwAEAAAAAAAAAAQAAwAAAwAAAAQEAgAAAAMEAAAAAAAB"
    "AwQAAgADAgEDAAQDAAAAAAIAAAAAAAAAAAAABAQAAAEEBAABAAAAAQQAAAAABAAAAAMCAAAAAAAD"
    "BAAAAAEEAwIAAAADAAAAAAAEAAIAAAMBAAADAAAAAAAAAgAAAAMCAAAEAgACAAADAAAAAwABBAAD"
    "AAIAAAAAAQAABAADAAAAAAQAAQABAAMAAwADAAAAAAAAAAMEAwADAwQBAAAAAAMAAAAAAAEDAAAE"
    "AQAAAAAAAgAAAQAAAAICAAIEAAABBAACAAABAgAAAQAABAIDAgAEAAMAAAAAAAEEAAMDBAADBAAA"
    "BAAAAAADAAABAwADAAAAAAMAAAQAAQIAAAAAAwICAAIAAAIAAAAAAQAAAAICAAMAAAEAAgQAAAAA"
    "AAQAAAAABAAAAAEAAAIAAAAAAAAAAAAAAAMABAAAAAADAgAAAAAABAAABAAAAwICAAIAAAACBAAD"
    "AAAAAAADAAABAAAAAQAAAAACAgAEAAAAAAAEBAAAAAAAAAIABAQBAAAAAAAEAQAAAAIAAQADAAAD"
    "BAADAAAEBAQAAAACAAAEAAAEAAAEAAIBAAAAAgECAAAAAAMCAAIEAgADAAMAAAADAAEAAQAAAAAB"
    "BAADAQAAAAAAAQADAAAEBAIAAAIAAQIDAAACAwAAAAMAAAAAAAAAAAQABAMAAAIDAAABAgEAAAAB"
    "AAEBAAIEAwAABAACAAQAAwEAAAAAAAAAAAABAQAAAAMBBAMAAwQABAMABAAAAwMDAQQEAAABAAEB"
    "BAAAAAAAAAABAAEDAQQAAAAABAICAAIEAAMAAAAAAwADAAQDAAECAQAAAAAAAAAAAAMCAgAAAAIA"
    "AAQEAAAAAAEAAAAAAgEAAQQAAAAEBAQDBAICAAADAgIAAQAAAQABAgQCAAABAwAAAwABAAQDAAAA"
    "AAAEAAAAAgABAAAABAAABAAAAAAAAwAEAAAAAAMAAwAAAAAAAAABAAAAAwMAAQMAAAAAAgABAAAA"
    "AAMAAQAAAQACBAAAAQAAAAECAgMAAAAAAAMAAAAEAgAAAwQCAAIAAAIAAAAAAAADBAAAAQAAAAAA"
    "AAEEAAAAAAAAAgQAAAADAAADAAAAAAAAAAAAAAIBAAEEBAAAAAAEAAAAAwABAAIBAwAAAAMEAAAA"
    "AgIDBAMAAAABAAEAAAMBAAMCAAAAAAADAAIBAAADAAAAAAABAQAAAAIAAAAEAAEAAAAAAAAABAAE"
    "AAAAAAMAAgEAAQMAAAAAAAACAAMBAgABAwAAAAAEBAAAAQADAAEAAAMBAAAAAQIAAwABAgECAQMA"
    "AAAAAAACAAAAAAEAAAAAAAAEAAAAAAMEAwABAAAEAAAAAAAAAAECAQEAAAAAAAAAAAACAAAAAQAE"
    "AAQAAAACAAQAAAAAAAAAAAEAAAABAAQBAwIAAAAAAAQCAAEBAAIAAgAAAAMEAAAEAAACAQEAAAAA"
    "AAAAAAQAAQQCAAQEAgMDAAQAAAMAAAADAAAEAAEAAwAEBAQDAAACAAEAAAAABAMDAAMAAAEAAAQA"
    "AgMAAwAABAABAAIDAAQAAAICAAIAAAAAAAIEAgAAAgAEAwIAAAABAAAEAQAAAwAAAAACBAECAQAA"
    "AwAAAwQAAwQDAAAAAAACAQQDAAAAAAAEAAAAAwMBAAAAAAQAAAAAAgIAAAADBAADBAAEAAQABAAA"
    "BAAAAwQBAAAAAAACAAACAAIAAAAEAAEABAAAAgAAAAAAAAAAAAEEAAAAAwAAAQIAAAMAAQACAwQE"
    "AQABAwAAAAAAAAAAAAMBAAAABAIAAAAAAAIEAAAAAgAAAwAEAwADAAACAAEDAwQEAwAAAAAAAAAD"
    "AwACAAIDBAAABAAEAAAAAAACAgACAgICAAAAAAAAAAADAAIDAAQBAAMAAgAAAgAAAAAAAAAAAQAE"
    "AwQAAQAAAAIBAgAAAAEAAAQAAAAAAAIAAAABAQAAAwABBAADAwABAAIAAAAAAQQBAgIABAAAAAQC"
    "AAACAgMCAwQDAAAAAAACAAABAAICAAAAAgIAAAAAAQIAAAAAAAABAAAAAAAAAAAAAAIBBAQEAAQA"
    "AgQBAAEAAAAAAAAEAwAAAAAABAAAAQABAAAAAgAAAAEAAAMBAgMAAQAAAQAAAAQAAAQAAAAAAAAA"
    "AAEAAgIAAAIAAAAAAAAEAgAAAAIBAAAAAAAAAAIEAAAAAgIAAAQAAAAAAwAAAgIAAAIABAMAAQAA"
    "AAAAAAADAAAAAAAAAAADAQADBAAAAwAAAAAAAAABBAACAQAAAAABAgADAAAAAAAAAgADAAMAAAID"
    "AAIAAAAEAAAABAAAAAAAAwABAQECAwAAAAEAAAAAAAQAAAAAAAEEAAMAAAAEAAAAAAIAAwECAAAA"
    "AQAAAAABAAAAAAAABAAAAAQABAECAAIBAAECAAAAAAADAAACAgAEAAQAAAAAAAMABAAAAQEABAAA"
    "BAEAAwMEAAMAAAQABAQDBAAAAAAAAwAAAgEEAAABAAAAAAAAAAIDAgAEAQABAwACAAAEAQQEAAIA"
    "AAADAAABAgMEBAAAAAAAAgACAAAABAQAAAABAAAAAAMDAwEAAAAEAAMABAAEAwIAAAQAAQAEAAAA"
    "AgAAAAAAAAEAAAAAAAAAAwEAAAEAAgACAAAAAQADAAAAAAEAAAAAAAAABAECAAAAAAIAAAQBAgIA"
    "AwAAAAIAAAMAAAAEAAIAAAIAAQACAAAAAAAAAAAAAAMCAAADAAEBAgAAAwAAAwADAwADAAQAAAAA"
    "AAIBAwAAAQAAAAEAAAABAAAAAAAEAAEAAAQAAgQDAgEEAgMCBAAAAQIAAgAAAgIAAAABAAQAAAAA"
    "AAAAAAEAAAAAAwQAAAAAAwAEAAAAAAADAAAAAAAEAAABBAAAAAAAAwQEAAAAAgQAAAAEAgAAAAAA"
    "AAEAAAECAAAABAIEAAAAAgAAAAECAgAAAAMDAgAAAAIBAAAEAAAAAAAAAAQAAAMAAAAAAwAAAQQA"
    "AAEDAQADAAMAAAAAAAAAAAEAAAIEAAICAQAAAAIAAAAAAAEBAAEAAAAAAAACAAMDAAEAAQAAAAAA"
    "AAADAAADAAAAAAEBAwMBAwEAAAIBAAQAAAAAAAADAAAAAAEAAAMAAAABAwMAAAAAAwAABAAAAAAA"
    "AwIAAAIDBAAEAAAAAwIAAgAAAAAAAAAAAAIAAAAAAwADAAMABAMAAgQAAwAAAwAAAAAEAgADAQAE"
    "AAQAAgAEAAAAAAADAAMAAAADAgACAQQAAAAEAAEABAAAAwEABAABAgAEBAABAwMEAAAAAQAEAgEE"
    "AAMBAAAAAAAAAAAEAAAAAAEAAAABAAAAAwAAAQIAAAMAAAAAAAAAAAAAAAACAAACBAACAAAAAAIA"
    "AAICAAEAAQAAAwMAAwEBAwAEAAMDAAQCAAIEAAABBAABBAEEAAECAQMEAAAAAAACAwADBAIBAwAB"
    "AAAAAwACAgMCAAMAAAAAAwMAAAQAAAQAAQAAAAAAAAMABAQAAwAAAAEAAgABAAAABAEAAAAAAAAC"
    "AQIAAAAAAAMAAwIAAQACAQMEAwQAAAAEAAMAAQAAAAADAQABAAQAAAABAQMBAAAEAQAAAAAAAAAE"
    "AAAAAAIEAAAEAAAAAAAEAwEAAAAAAAIAAgAAAwEAAAEAAgAAAAMAAAQEAwAAAAADAQABAwAAAAAB"
    "AwADBAAEAQAAAwAABAAABAAAAAAAAAABAAAAAAMCAAAAAgEAAAQDAQAAAAMDAAAEAAIABAAAAAAA"
    "AQMEAAAAAAAAAAAAAAEEBAAEAAQDAAAAAAAAAgAAAAMAAwAAAAEAAAAAAgAAAQAAAgAEAAADBAAA"
    "AwABAAAAAwADAAICAAIAAAICAgMEAgAAAAAAAQACAAQBBAAAAQEBAAAAAAIAAAAAAgACAAIAAAAA"
    "AQAABAIDAAAAAAAAAAAAAAAEAAAAAAABAQAAAAAEAAAAAwABAwAAAAIEAAAABAEAAgMCAwACAAAC"
    "AAADAAAAAwAAAAMAAwMAAgACAAAAAAEDBAQAAwIDAAAAAAQCAgADAAADAgAAAAAAAwAAAAMBAQEA"
    "AwEAAwABAAAAAAMCAAAAAAADAAAABAQDBAAABAEAAwAAAAQEAAAAAwAAAgIBBAACAAABAAQAAAAD"
    "AAQABAICAAAEAQMAAAACBAEAAAIAAAMEAAAABAADAAAAAAIAAAMAAQAAAAABAAIAAAACAwMDAAAA"
    "AgACAAIEAAAAAAEEAAEAAAMDAAQEBAEAAAAAAAAAAAEAAgAEAAQAAAAEAAMABAABAQMAAQADAAID"
    "AAAAAAMCAgEAAwQAAgIAAAAEAAEAAAAAAAAABAAAAAAAAAQAAAAEAAAABAAAAAAAAAAAAAAAAAAA"
    "AAAEAwMAAQMAAwQAAQABAwACAAMAAAAAAAADAQAEAgAAAgIBAAQBBAAAAAAAAAQAAQAEAgAEAAIC"
    "AAIEAAIAAgAAAAADAAAABAQAAAACBAEEAwIABAACAAAAAAMABAABAAAAAAMAAAQAAAABAAMAAAAA"
    "AgACAAMAAAAAAwAAAAIAAAAAAAAAAAMEAAQEAAIAAQAAAAQDBAAAAAQABAMAAQQAAQAAAAEEAAMD"
    "AQAABAADAAAAAAABAgAAAAAABAIAAAABAAAABAABAgECAwMAAAACAgEABAABAAAAAgEBAAAEBAAC"
    "AAAAAgEAAAMAAAACAAAAAgMAAAAAAAQBAAAAAAACAQMCAAABAAADAAADAwABAAIAAAADAAADAQAA"
    "AAAABAACAAAAAAIAAAAABAMDBAQAAAAAAAQBAAQAAAAAAAAAAQAAAAEEAAMABAEAAAAEAgAAAAMA"
    "AAAAAgMCAgIAAAAAAgAAAAAAAAMAAAAAAAEAAAAAAgMBAAMAAAAABAMEAAQAAAMAAwACBAAEAAAB"
    "AAAAAAACBAQABAAEAgQAAAAEAQMDAAMAAAIEAQAEBAADAQIABAEDAAAAAgQABAADAAAAAgACBAMB"
    "AAMDAAAAAAAAAAIDAAAAAAIABAADAAAAAQAAAAAAAAAEAQAAAgABAAMDBAIBAAAABAADAAMEAwQA"
    "AAQCAAEAAwMAAAQBAAACAAABAAEAAAQCBAMBAgAAAAAAAAAABAQCAwMABAAAAAAAAAAAAAAAAQME"
    "AAAAAQAABAACAAMCAwEBAAACAgAAAgEAAAADAAAEBAAAAAAAAAABAAABAwMAAAMCAwAEAwIAAAQA"
    "BAICAAEBAAIAAAACAgIBAAAAAgQCAgAAAQQAAAAAAAAAAAMEAAADAwQABAACBAQAAwQAAQEDAQAA"
    "BAAAAAAAAwAAAAACAAMAAgMEAwEAAAAAAAEDAAAAAAIBAAQAAAMAAAMABAAEAAEEAwMAAAABBAAE"
    "AAIEAwAAAAAAAAMAAgQAAAMAAAEAAQIAAAMDBAAABAAAAAMAAAAEAAAEAAMAAAAAAAAAAAMAAAAE"
    "AAABAwAAAQAAAAEEAAAAAAIAAQAEAAAAAAADAAMAAAQDAAAAAgQCAgEAAAIBAAAAAAADBAIAAAMA"
    "AAQAAQQAAAACAAAAAAMAAgAAAQMAAAAAAQADAAIAAAAAAgAABAAAAAQEBAAEAQQAAwABAAACAAAA"
    "AAAAAAAAAAADAAAEAAABAgADAAIAAgEDAAADAAAAAAADAwQAAAMBAAAAAAAAAAAAAgABAQADAQQA"
    "BAAAAwAAAAABAAAAAAIDAAAAAwAEAAAAAQAAAAAAAwAAAAIDAAAAAwADAAQAAAEAAAECAAIABAAA"
    "BAAABAACAAMAAQAAAAIAAgIAAgAAAAQAAQACAAACAAABAAEBAAIDAAIABAAAAwEAAgMAAAAAAAMA"
    "BAACBAAAAAAABAABBAAEAAAAAQQAAQAAAAAEAgAAAAAAAwADAAAAAAAAAAMAAAAAAAEAAAAABAEA"
    "AAAEAgIAAAIAAAAAAAAAAAAAAAEEAAADAAAAAAEAAwAAAAMEAgAAAAAAAAAAAAIEAAEAAQAABAAA"
    "BAEAAAQAAwAAAwABAAIDAwQEAAAAAwQAAAQABAMAAAECAgACAAIDAAAAAQIEAAQABAQDAAAAAAAA"
    "AAAAAAAAAwABAwAAAQADAwIAAAAAAQABAAAAAAEABAQBAwABAAADAgAEAAIAAAMABAEAAAEAAQAA"
    "BAMAAwQCAwMAAQMCAwQAAwAAAAEABAAAAAEAAgEAAAAAAAAAAAAAAAAAAgAEAQAAAAEAAAAEAwAA"
    "AQIABAMEAAABAAMAAgEEAAIAAAEEAAABAAABAQAAAAAAAgIAAAAAAAADAgABBAMEAgACBAACBAQA"
    "AgADAAACAgQAAwADAwAEBAQAAAEBAAAABAECAAAAAAAABAACAAAEBAAAAAADAAAEAAMAAAIBAAAA"
    "AAQAAQAABAAAAAACAAEDAwAEBAAAAAAAAAACAQAAAAAEAAIAAAADAAAAAAIAAwAAAAEEBAAAAgAD"
    "AAAAAgEAAAQAAAEAAAAAAAIEAAMAAwQABAACAAEBAAEAAAEABAAAAAICBAQAAQAAAgIEAAAAAAAA"
    "AAAAAAAABAIBAAAAAgIAAAACAQAAAAABAAAAAAQEAgAEAAABAAAAAAAAAAEAAAMCAwAEBAMDBAAA"
    "AAABAAABAAEBAAABAwAAAAABAAABAwMAAAABAAMEAAAAAgAAAAQAAAACAAMAAAAAAAAAAAQAAAQD"
    "AAAABAABAAIAAAIAAAAAAAICAwACAwABAAAAAAQAAwADAgAAAAAAAgEABAIAAAAAAAABBAAAAAIC"
    "AAQAAAQAAAEAAwMDAAAAAQAEBAAAAAEAAAEBAAAAAgAAAwIABAADAAAEAgAAAAAABAAAAAAAAAAC"
    "AAQAAgAEAwAAAAAEAAMEBAEAAQACAAAEAAAABAAAAAAAAAAEAQQAAAQEAAQAAgAAAQEAAQAAAAQE"
    "AAABAAAAAAQABAAEAQAABAACAwACBAQEAAAAAQEAAQABAAAAAAAAAAAAAQAAAQAAAAAEAAACAAAA"
    "BAACAAEAAAAAAAMAAAIAAAMEAQAAAAIBAAIBAAAABAECAAAAAAAAAAABAAMBAAAAAwQAAgAAAwAA"
    "AwAEAQQAAwAAAQQAAwQAAAABAAABAAAEAAQAAAACAAABAAAAAAAAAQIAAAABAAAAAAICAAACAAIA"
    "AAADAgMCAAABAAAAAwACAAMABAAAAAAAAAAAAAAAAAIAAAAAAAQBAAAAAAECAQMBAAAAAAACAAAD"
    "AAAAAAQCAAQBAAACAAAAAAMAAwIAAgMAAAABAwMDBAAABAAEAAAAAAEBAAQCAQAEAAQABAIAAAID"
    "AAEAAQAAAAACAAQAAAABAAADAQECAAAAAAQAAAMABAACAAAAAAQAAAAAAAAAAQEDAAABAwQDAwIA"
    "BAAAAQADAAAAAgAEAwAABAABAQAABAABAAQAAgAAAAAAAAQAAAMBAAACBAAEAAEEAAAABAAABAAA"
    "AAAABAMDAAEBAAAAAAAEAgMAAAAEAgADAAACAgAAAAMAAAQBAQAAAQAEAgAAAAMDAAAAAAABBAAA"
    "AAAAAwQBAAIAAAABAAIAAAIABAMAAAAEAwMAAAABAAAAAwECBAAABAAAAAACAAAAAAAAAAAEAQIB"
    "AAAABAMAAAQCAwEBAgAAAAQAAQAAAAABAAAAAAIAAwACAwECAQAAAgMCAwAEAAAEAQQAAAAAAwAA"
    "AAMAAAMAAAAABAAAAAAAAAMAAAMEAAAAAAAEAAAAAAAAAAQAAwECAAQAAAAAAgAAAAAAAAAAAAAA"
    "AAAEAAADAwAAAAMCAAIAAAAAAwAAAgADAAACAAADAAAAAAMBAAEBAAECAAADAAAEAQMDBAACAAAC"
    "AAABAAACAAQAAAAAAQAAAAAAAQABAwQAAAQCAAAAAwMAAQADAAMAAAMAAAIAAAAAAAAAAAEEAAAA"
    "AAMAAAMEAAACAAAAAAMAAwIAAQMAAgIAAAIAAQAAAAAABAMAAAAAAgEAAAABAQEBAAQAAgQDAAAA"
    "BAMAAAEAAAAAAgIAAwMAAAAABAIAAAADAAECAgIAAAEBAAMBAAQAAgAAAAIAAAIAAAAAAAQEAAAD"
    "AQEEAQIDAAACAAACAAIEAAECAAAAAgMCAwACAAABAwAAAwAAAAAABAAEAAQDAAAAAAABAQEBAAAE"
    "AAAAAwAAAgAAAAADAAECAQMAAAABAAACAAAAAAAAAwMAAAIAAAIAAAEBAAIEAAAEAAAAAAAAAAMA"
    "AQQAAAMEAAMAAwMAAQAAAAAAAAMEAAQCAAIDAAMDBAQAAAAEAAEAAAMCAQACAgAAAAEDAAQAAwAA"
    "AAAAAQQAAAICBAMAAAEAAAAAAAQDAAAAAQAAAQADAAADAAAAAAAAAQAABAAAAAAAAQADAgICAQIA"
    "AAIBAAEAAwAAAAAAAAADAwAAAAAABAIAAAAAAAAEAAMABAAAAAAAAAQAAwQABAAAAAAAAAAAAwED"
    "AAMAAAAAAAAABAMAAAAAAwEAAgABAAAAAQAAAAACAAAAAAAEAQABAAABAQAAAQAAAAMAAgABAAMA"
    "AAAABAAEAQAAAAMABAAAAAEAAQAAAwQDAAACAAQEAAACAAAEBAAAAAMBAAABAAACAAAAAAQAAAAB"
    "AAADAQIBAAADAAEAAQAAAgMBAAADAAIDAAQAAAAAAQEBAQAAAgMAAAACAAAEAwABAAAAAAAEAAAD"
    "AAEEAwEAAQAAAQACAAEAAAMAAQMAAgAAAAIAAAQAAAAAAAIDAAAAAAA="
)

_RANK_CACHE = None


def _get_rank() -> np.ndarray:
    """rank[i]: which of the 5 nearest negatives the reference picks per row.

    Must reproduce the reference's jax.random draws bit-exactly; compute on
    the CPU jax backend when available, else use the embedded constant
    (generated the same way).
    """
    global _RANK_CACHE
    if _RANK_CACHE is not None:
        return _RANK_CACHE
    try:
        import jax
        import jax.numpy as jnp

        cpu = jax.devices("cpu")[0]
        with jax.default_device(cpu):
            k1, k2 = jax.random.split(jax.random.key(1))
            coin = jax.random.uniform(k1, (B,)) < 0.5
            rank = jnp.where(coin, 0, jax.random.randint(k2, (B,), 0, HARD_RANK))
            r = np.asarray(jax.device_get(rank)).astype(np.uint8)
    except Exception:
        r = np.frombuffer(base64.b64decode(_RANK_B64), dtype=np.uint8)
    assert r.shape == (B,)
    _RANK_CACHE = r
    return r


_NC_CACHE = None


def _build_nc():
    import os as _os

    NBUF = int(_os.environ.get("K_NBUF", "8"))

    import concourse.mybir as mybir
    import concourse.tile as tile
    from concourse import bacc

    F32 = mybir.dt.float32
    BF16 = mybir.dt.bfloat16
    FP8 = mybir.dt.float8e4
    PM = mybir.MatmulPerfMode

    nc = bacc.Bacc()
    # at8[p, t*M + i] = 32*ahat.T[t*128+p, i]   (core's anchor slab)
    at8 = nc.dram_tensor("at8", [128, 2 * M], FP8, kind="ExternalInput").ap()
    # pt8[p, k*2*GW + t*GW + j] = 16*phat.T[t*128+p, k*GW+j]: column chunks of
    # GW cols, both k-tiles packed per chunk, so each chunk DMAs separately
    pt8 = nc.dram_tensor("pt8", [128, 2 * B], FP8, kind="ExternalInput").ap()
    # 8 direct candidates (f32) + 8 tree candidates (bf16) per (row, rb)
    cd = nc.dram_tensor("cd", [128, RB * 8], F32, kind="ExternalOutput").ap()
    ct = nc.dram_tensor("ct", [128, RB * 8], BF16, kind="ExternalOutput").ap()

    with tile.TileContext(nc) as tc:
        with (
            tc.tile_pool(name="ops", bufs=1) as opsp,
            tc.tile_pool(name="evac", bufs=NBUF) as evacp,
            tc.tile_pool(name="tree", bufs=NBUF) as treep,
            tc.tile_pool(name="out", bufs=2) as outp,
            tc.tile_pool(name="ps", bufs=2, space="PSUM") as psp,
        ):
            # at8 packed [p, rb*256 + t*128 + m]; rb0's slice lands first so
            # the pipeline starts as soon as column chunk 1 arrives
            # PE warmup fodder: ramp the p-state during the DMA head
            warm = opsp.tile([128, 128], FP8, tag="warm")
            nc.gpsimd.memset(warm, 0.0)
            # rb0's anchor slice is its own tile so its matmuls don't wait
            # for the full anchor DMA (deps are tile-granular)
            a0 = opsp.tile([128, 256], FP8, tag="a0")
            nc.sync.dma_start(a0, at8[:, :256])
            a_sb = opsp.tile([128, 2 * M - 256], FP8)
            # chunk 1 lands as two separate tiles (tile-granular DMA deps)
            # so granule 1's first matmuls start on the first half
            HW = GW // 2
            pc1a = opsp.tile([128, 2 * HW], FP8, tag="pc1a", name="pc1a")
            nc.sync.dma_start(pc1a, pt8[:, 2 * GW : 2 * GW + 2 * HW])
            pc1b = opsp.tile([128, 2 * HW], FP8, tag="pc1b", name="pc1b")
            nc.sync.dma_start(pc1b, pt8[:, 2 * GW + 2 * HW : 4 * GW])
            nc.sync.dma_start(a_sb, at8[:, 256:])
            pc1h = [
                pc1a.rearrange("p (t n) -> p t n", t=2),
                pc1b.rearrange("p (t n) -> p t n", t=2),
            ]
            pcol = [None, None, None, None]
            for k in (2, 3, 0):
                pc = opsp.tile([128, 2 * GW], FP8, tag=f"pc{k}", name=f"pc{k}")
                nc.sync.dma_start(pc, pt8[:, k * 2 * GW : (k + 1) * 2 * GW])
                pcol[k] = pc.rearrange("p (t n) -> p t n", t=2)
            a0v = a0.rearrange("p (t m) -> p t m", t=2)
            lhsT_all = a_sb.rearrange("p (r t m) -> p r t m", r=RB - 1, t=2)

            for rb in range(RB):
                lhsT = a0v if rb == 0 else lhsT_all[:, rb - 1]
                evs = []
                hs = []  # per-granule halved [128, GW//2] bf16

                def granule(gi, lhsT=lhsT, rb=rb):
                    ps = psp.tile([128, GW], F32, tag="ps")
                    if rb == 0 and gi == 1:
                        # warmup matmuls: ramp PE while DMAs land (the real
                        # matmuls below overwrite with start=True)
                        wv = warm.rearrange("p (t n) -> p t n", t=2)
                        for _ in range(2):
                            nc.tensor.matmul(
                                ps[:64, :64],
                                wv,
                                wv,
                                start=True,
                                stop=True,
                                perf_mode=PM.DoubleRow,
                            )
                    for q in range(GW // MMW):
                        if gi == 1:
                            rhs = pc1h[q // 2][
                                :, :, (q % 2) * MMW : (q % 2 + 1) * MMW
                            ]
                        else:
                            rhs = pcol[gi][:, :, q * MMW : (q + 1) * MMW]
                        nc.tensor.matmul(
                            ps[:, q * MMW : (q + 1) * MMW],
                            lhsT,
                            rhs,
                            start=True,
                            stop=True,
                            perf_mode=PM.DoubleRow,
                        )
                    return ps

                def direct(gi, rb=rb):
                    c8 = outp.tile([128, 8], F32, tag="c8")
                    nc.vector.max(out=c8, in_=granule(gi))
                    nc.sync.dma_start(cd[:, rb * 8 : (rb + 1) * 8], c8)

                def evac(gi, halve=True):
                    ev = evacp.tile([128, GW], BF16, tag=f"e{len(evs)}")
                    nc.scalar.copy(ev, granule(gi))
                    evs.append(ev)
                    if halve:
                        h = treep.tile([128, GW // 2], BF16, tag=f"h{len(hs)}")
                        nc.vector.tensor_max(
                            h, ev[:, : GW // 2], ev[:, GW // 2 :]
                        )
                        hs.append(h)

                # ACT granules land in slot order S1,S2,S1 (no in-block
                # bubble); the DVE-mined granule fills S2 after ACT's second
                # drain.  For the last block the DVE granule goes first so
                # its (late-slot) Max8 is off the tail's critical path.
                if rb < RB - 1:
                    evac(1)
                    evac(2)
                    evac(3, halve=False)
                    direct(0)
                else:
                    direct(0)
                    evac(1)
                    evac(2)
                    evac(3, halve=False)
                h = treep.tile([128, GW // 2], BF16, tag="h2")
                nc.vector.tensor_max(
                    h, evs[2][:, : GW // 2], evs[2][:, GW // 2 :]
                )
                hs.append(h)
                # merge chain (bucket 16)
                m1 = treep.tile([128, GW // 2], BF16, tag="m1")
                nc.vector.tensor_max(m1, hs[0], hs[1])
                m2 = treep.tile([128, GW // 2], BF16, tag="m2")
                nc.vector.tensor_max(m2, m1, hs[2])
                q1 = treep.tile([128, GW // 4], BF16, tag="q1")
                nc.vector.tensor_max(q1, m2[:, : GW // 4], m2[:, GW // 4 :])
                q2 = treep.tile([128, GW // 8], BF16, tag="q2")
                nc.vector.tensor_max(q2, q1[:, : GW // 8], q1[:, GW // 8 :])
                t8 = outp.tile([128, 8], BF16, tag="t8")
                nc.vector.max(out=t8, in_=q2)
                nc.sync.dma_start(ct[:, rb * 8 : (rb + 1) * 8], t8)

    nc.compile()
    return nc


def _get_nc():
    global _NC_CACHE
    if _NC_CACHE is None:
        _NC_CACHE = _build_nc()
    return _NC_CACHE


def _prep(x: np.ndarray):
    """Host prep: normalize, scale, transpose, interleave, fp8-quantize."""
    x = np.ascontiguousarray(np.asarray(x, dtype=np.float32))
    assert x.shape == (B, 2, D)
    x0 = x[:, 0, :]
    x1 = x[:, 1, :]
    na = np.sqrt(np.sum(x0 * x0, axis=1, keepdims=True))
    np_ = np.sqrt(np.sum(x1 * x1, axis=1, keepdims=True))
    ahat = x0 / np.maximum(na, 1e-12)
    phat = x1 / np.maximum(np_, 1e-12)

    a8 = (ASCALE * ahat).astype(ml_dtypes.float8_e4m3)
    p8 = (PSCALE * phat).astype(ml_dtypes.float8_e4m3)
    # the device sees the fp8-rounded values; use them for the exact diag
    a8f = a8.astype(np.float32)
    p8f = p8.astype(np.float32)
    sii_dev = np.einsum("ij,ij->i", a8f, p8f)  # approx of diag psum value

    aT = np.ascontiguousarray(a8.T)  # [D, B]
    pT = np.ascontiguousarray(p8.T)  # [D, B]

    pt8 = np.empty((128, 2 * B), dtype=ml_dtypes.float8_e4m3)
    for k in range(NG):
        if k == 1:
            # chunk 1 is stored as two half-chunks of GW//2 cols each
            hw = GW // 2
            for h in range(2):
                base = k * 2 * GW + h * 2 * hw
                c0 = k * GW + h * hw
                for t in range(2):
                    pt8[:, base + t * hw : base + (t + 1) * hw] = pT[
                        t * 128 : (t + 1) * 128, c0 : c0 + hw
                    ]
        else:
            for t in range(2):
                pt8[:, k * 2 * GW + t * GW : k * 2 * GW + (t + 1) * GW] = pT[
                    t * 128 : (t + 1) * 128, k * GW : (k + 1) * GW
                ]

    in_maps = []
    for c in range(NCORES):
        # at8[p, rb*256 + t*128 + m] = aT[t*128+p, c*M + rb*128 + m]
        at8 = np.empty((128, 2 * M), dtype=ml_dtypes.float8_e4m3)
        for rb in range(RB):
            for t in range(2):
                lo = rb * 256 + t * 128
                at8[:, lo : lo + 128] = aT[
                    t * 128 : (t + 1) * 128,
                    c * M + rb * 128 : c * M + (rb + 1) * 128,
                ]
        in_maps.append({"at8": np.ascontiguousarray(at8), "pt8": pt8})
    return in_maps, ahat, phat, sii_dev


def _epilogue(cands: np.ndarray, ahat, phat, sii_dev) -> np.float32:
    """cands: [B, 16] raw psum-scale candidate values, unsorted."""
    rank = _get_rank()

    order = np.argsort(-cands, axis=1)
    csort = np.take_along_axis(cands, order, axis=1)  # desc [B, 24]

    # drop the self-match: closest candidate to the (device-precision) diag
    # value, if within the fp8 noise band
    TOL = 8.0  # psum units; fp8 dot noise sigma ~1.7, bf16 evac ~0.5
    diff = np.abs(csort - sii_dev[:, None])
    kstar = np.argmin(diff, axis=1)
    hit = diff[np.arange(B), kstar] < TOL
    # shift left past the dropped slot where hit
    idx = np.arange(8)[None, :] + (
        hit[:, None] & (np.arange(8)[None, :] >= kstar[:, None])
    )
    top = np.take_along_axis(csort, idx, axis=1)  # [B, 8] diag-free

    s_sel = top[np.arange(B), rank] / SS  # = <ahat_i, phat_neg>
    ra = np.sum(ahat, axis=1)
    pos_d2 = np.sum(np.square(ahat - phat + EPS), axis=1)
    neg_d2 = 2.0 - 2.0 * s_sel + 2.0 * EPS * ra + D * EPS * EPS
    return np.float32(np.mean(np.maximum(pos_d2 - neg_d2, 0.0)))


def kernel(x: np.ndarray, _want_timing: bool = False):
    """x: [8192, 2, 256] float32 -> scalar float32 loss (0-d ndarray)."""
    from concourse.bass_utils import run_bass_kernel_spmd

    in_maps, ahat, phat, sii_dev = _prep(x)
    nc = _get_nc()
    res = run_bass_kernel_spmd(nc, in_maps, list(range(NCORES)))

    cands = np.empty((B, 16), dtype=np.float32)
    for c in range(NCORES):
        cdv = res.results[c]["cd"]  # [128, RB*8] f32
        ctv = res.results[c]["ct"].astype(np.float32)  # [128, RB*8]
        for rb in range(RB):
            rows = slice(c * M + rb * 128, c * M + (rb + 1) * 128)
            cands[rows, 0:8] = cdv[:, rb * 8 : (rb + 1) * 8]
            cands[rows, 8:16] = ctv[:, rb * 8 : (rb + 1) * 8]

    out = _epilogue(cands, ahat, phat, sii_dev)
    if _want_timing:
        return np.asarray(out), res, cands
    return np.asarray(out)


if __name__ == "__main__":
    rng = np.random.default_rng(0)
    x = rng.standard_normal((B, 2, D)).astype(np.float32)
    print(kernel(x))


# revision 67
# speedup vs baseline: 1.1383x; 1.1314x over previous
"""Trainium2 Bass kernel for nn_LossFunction_103079215159 (triplet-style loss
with online hard-negative mining).

Math (B=8192, D=256; x[:,0]=anchors, x[:,1]=positives):
  a = l2norm(x0), p = l2norm(x1)
  dist[i,j] = ||a_i - p_j + eps||, self-match excluded
  top5 smallest per row -> pick rank[i]-th (RNG-derived, data-independent)
  loss = mean relu(||a_i-p_i+eps||^2 - ||a_i-p_neg+eps||^2)

Reduction: with s[i,j] = <32*a_i, 16*p_j> = 512*cos_ij, smaller distance ==
larger s. Device mines the top-8 LARGEST s per row (diag included; dropped on
host by value-match against the exactly-known s_ii).

Device program (per core, 8-way data parallel over anchor rows):
  - inputs are host-prepped fp8e4 operands, pre-normalized, pre-scaled and
    pre-transposed into the PE's [contraction-partition, 2 k-tiles, cols]
    layout, so the device does NO norms / scaling / transposes.
  - PE: fp8 DoubleRow matmuls (contraction 256 in one instruction, 0.5
    cycles/out-col, 512 out-cols each) fill [128, 2048] f32 PSUM granules;
    4 granules per 128-row block, 2 PSUM slots in flight.
  - PSUM egress is the bottleneck (only ACT and DVE can read PSUM; Pool
    cannot touch it at all, and TensorTensor may read at most one PSUM
    operand):
      granules 1,2,3 -> ACT copies to SBUF bf16 (slot order a,b,a)
      granule 0      -> DVE Max8 straight from PSUM -> 8 exact candidates
  - DVE folds the three bf16 slabs (tensor_max at the 2x DVE rate) down to
    256 cols (bucket 16), then Max8 -> 8 tree candidates. 16 candidates
    per (row, block) stream out via per-block DMAs.
Host: normalize, fp8 prep, exact s_ii / pos_d2, diag drop-by-value, rank
select, relu + mean. All O(B*D) numpy, vs the device's O(B^2*D/8)/core.
"""

import base64

import ml_dtypes
import numpy as np

B = 8192
D = 256
NCORES = 8
M = B // NCORES  # 1024 anchor rows per core
RB = M // 128  # 8 row blocks per core
GW = 2048  # psum granule width (4 banks)
NG = B // GW  # 4 granules per row block
MMW = 512  # out cols per DoubleRow matmul
NACT = 3  # granules per row block evacuated by ACT (rest mined by DVE Max8)

ASCALE = 32.0  # anchors uploaded as 32*ahat  (32*16 = 512 = 256*s_scale)
PSCALE = 16.0  # positives uploaded as 16*phat
SS = ASCALE * PSCALE  # psum value = SS * <ahat_i, phat_j>

EPS = 1e-6
HARD_RANK = 5

# rank[i] in {0..4}: which of the 5 nearest negatives to use per row.
# Reproduces exactly:
#   k1, k2 = jax.random.split(jax.random.key(1))
#   coin = jax.random.uniform(k1, (8192,)) < 0.5
#   rank = jnp.where(coin, 0, jax.random.randint(k2, (8192,), 0, 5))
_RANK_B64 = (
    "AAIEAAAAAAAAAAIAAwAAAAAAAAAAAAMAAAIAAAMABAAAAAAAAwACAAABAAQCBAADAAACAgAEAwAC"
    "AAMEAAAAAwEEAQMAAAIAAgAAAAAAAAAEAAQAAwAABAECAAIAAAAAAgADAAACAwQABAAAAgMAAgAE"
    "AwAAAgACAAECAAEAAAECAQEBAAAABAACBAAAAAAAAAEAAAAEAQAAAAIAAgADAAEAAAAAAQAAAQME"
    "AgAAAAEEAAAAAAMAAQAAAAAEAAAEAQAAAAAAAAAAAAAAAAADAQQAAAAAAgABAAAAAAADAAADAAQA"
    "AAAAAwMAAAAEAAAAAAAAAAEAAAMAAAAAAAQAAAACAgAEAQAAAAABAAADAgABAAIAAAAAAwQCAAAD"
    "AgAAAAADAgAAAQAABAAABAAAAAAAAAIAAAEABAADAAAAAAAEAAAAAQEBAAAAAAMAAAIAAAAAAAMA"
    "AwIDAAEAAQQAAAIAAAEEAAECAAIAAAEAAAADAAIAAQICAAABAgAAAQAAAAIAAAADAAEDBAAAAQEA"
    "AgAAAAAEBAAAAAEAAgECAAIEAAAABAAEAQIABAAAAAAAAAAAAAMBAQAAAAMCAgADAAIDAwQDBAAE"
    "AAAAAAAAAAEAAAEAAwMAAAAAAAAAAAABAAAAAAAAAAEAAAADAgMAAAMAAAAAAAMAAQAAAAAAAgAA"
    "BAAAAAMBAQABAAAAAAAAAAIAAwAAAgAEAwABAAAAAAAAAAAAAAIAAgABAgAEAAABAQIAAgIDAgAE"
    "AAAAAAAAAQAABAAEAAAAAAAAAQIAAgAAAAMAAQACAAAAAAADAAQAAQABBAAEAAMABAABAQADAQAA"
    "AgABAgAEAAIAAAAAAgAAAwAAAwAAAAAEAAAAAAEAAAAAAAIEAAAAAgAABAEAAgAAAAAAAAEAAAAC"
    "AAECBAADAAAAAQAAAAIAAAAAAgMAAAAAAQAAAAQAAAAAAAMEAwEAAgEAAAAAAAAABAADAQIDAAAA"
    "AAEAAwAAAgAAAAEAAgAAAAAAAgAAAAAABAAEAAACAAIAAAQAAgADAAEAAAQAAAACAAECAwIEAAAA"
    "BAQAAAQABAMAAAQAAwIAAQMAAAQAAAACAAAEAAAABAAAAAAAAAMBAAEAAAQDAAAAAAQDAAAAAAIA"
    "AAAEAwACAAQAAgACAAACAQQAAAQDAgQDAQAAAAAEAAADBAECBAAEAAEBAAAAAAEAAgAAAwAAAgAB"
    "AwAAAgAEBAAAAAIEAAAAAwACAAIBAAABAwQAAQAAAAQAAAAAAAIAAAEBAAIAAAAAAAEAAAAAAAEB"
    "AAAAAgACAAAAAAMAAwAAAAAABAMABAMAAQQBAAQCAAEDAAAAAAIAAAAEAAMDAAAEAAEAAQAAAAAA"
    "AAICBAABAQQEAAAAAAQAAQABAAEEAAACBAAAAAMAAAAABAAAAAEBAAICAAIAAAAAAAAEBAAAAAMC"
    "AAQDAAABAAQCAAEAAAAABAQEAAIBAAAAAgAEAAEAAAIEBAACAAIAAAAABAMDBAQAAAAAAAIAAgAA"
    "AAACAAABAwMDAAAAAAAAAAACAQAAAwAAAAAEAAAAAAMAAAAAAgMAAAICAAMAAAAEAAAAAAABAAAA"
    "AAABAAAAAAMAAAEEAAIDAAEBAAQAAAMCAAAAAAAEAAACAAMAAAACAwAAAwAEAAAAAAQAAwABAAAC"
    "AwAAAAEABAQBAAIAAAIAAwAEAAEAAAACAgAAAAEEAAQAAAADAAMDAAQDBAABBAACAwAAAAAEAAMA"
    "AgQABAIAAAAEAAQCAQMAAAIBAAIAAAQEAAACAAEAAAAAAAEAAAABAAEAAAAABAAAAAAABAADAAAA"
    "BAABBAABAAADAAAAAAAAAAAAAQAAAAAAAAMAAQAAAQACAAAAAAACAAMAAAMAAwIBAAAABAAAAAMA"
    "AAAAAAABAAABAQIBAAAAAgAAAAAEAAAAAAQAAAAAAwAAAAAAAgAAAAAAAAAAAAACAgAAAAABBAAA"
    "AwACAAEDAAAAAAQAAQACAAAEAAAAAgAAAAIAAAMBAAAAAAIEAwAAAAQAAAMAAAMAAAAAAAAAAAMC"
    "BAQAAAMAAAEBAQAAAAAAAAIAAAMAAAMAAAAAAAIABAAAAAABAgAAAAAEAAQCAAIAAAIDAAMBAAAA"
    "AwAAAQADAwABAAADAAAEAwAAAAAABAMAAAEAAAAAAAAAAAAAAAAAAAAAAAACAAAAAAICAgACAAMA"
    "AAACAwAAAAIAAQAAAAAEAQAAAgAEAAEAAwAEAAAAAAAAAAQAAwAAAwAAAAQEAgAAAAMEAAAAAAAB"
    "AwQAAgADAgEDAAQDAAAAAAIAAAAAAAAAAAAABAQAAAEEBAABAAAAAQQAAAAABAAAAAMCAAAAAAAD"
    "BAAAAAEEAwIAAAADAAAAAAAEAAIAAAMBAAADAAAAAAAAAgAAAAMCAAAEAgACAAADAAAAAwABBAAD"
    "AAIAAAAAAQAABAADAAAAAAQAAQABAAMAAwADAAAAAAAAAAMEAwADAwQBAAAAAAMAAAAAAAEDAAAE"
    "AQAAAAAAAgAAAQAAAAICAAIEAAABBAACAAABAgAAAQAABAIDAgAEAAMAAAAAAAEEAAMDBAADBAAA"
    "BAAAAAADAAABAwADAAAAAAMAAAQAAQIAAAAAAwICAAIAAAIAAAAAAQAAAAICAAMAAAEAAgQAAAAA"
    "AAQAAAAABAAAAAEAAAIAAAAAAAAAAAAAAAMABAAAAAADAgAAAAAABAAABAAAAwICAAIAAAACBAAD"
    "AAAAAAADAAABAAAAAQAAAAACAgAEAAAAAAAEBAAAAAAAAAIABAQBAAAAAAAEAQAAAAIAAQADAAAD"
    "BAADAAAEBAQAAAACAAAEAAAEAAAEAAIBAAAAAgECAAAAAAMCAAIEAgADAAMAAAADAAEAAQAAAAAB"
    "BAADAQAAAAAAAQADAAAEBAIAAAIAAQIDAAACAwAAAAMAAAAAAAAAAAQABAMAAAIDAAABAgEAAAAB"
    "AAEBAAIEAwAABAACAAQAAwEAAAAAAAAAAAABAQAAAAMBBAMAAwQABAMABAAAAwMDAQQEAAABAAEB"
    "BAAAAAAAAAABAAEDAQQAAAAABAICAAIEAAMAAAAAAwADAAQDAAECAQAAAAAAAAAAAAMCAgAAAAIA"
    "AAQEAAAAAAEAAAAAAgEAAQQAAAAEBAQDBAICAAADAgIAAQAAAQABAgQCAAABAwAAAwABAAQDAAAA"
    "AAAEAAAAAgABAAAABAAABAAAAAAAAwAEAAAAAAMAAwAAAAAAAAABAAAAAwMAAQMAAAAAAgABAAAA"
    "AAMAAQAAAQACBAAAAQAAAAECAgMAAAAAAAMAAAAEAgAAAwQCAAIAAAIAAAAAAAADBAAAAQAAAAAA"
    "AAEEAAAAAAAAAgQAAAADAAADAAAAAAAAAAAAAAIBAAEEBAAAAAAEAAAAAwABAAIBAwAAAAMEAAAA"
    "AgIDBAMAAAABAAEAAAMBAAMCAAAAAAADAAIBAAADAAAAAAABAQAAAAIAAAAEAAEAAAAAAAAABAAE"
    "AAAAAAMAAgEAAQMAAAAAAAACAAMBAgABAwAAAAAEBAAAAQADAAEAAAMBAAAAAQIAAwABAgECAQMA"
    "AAAAAAACAAAAAAEAAAAAAAAEAAAAAAMEAwABAAAEAAAAAAAAAAECAQEAAAAAAAAAAAACAAAAAQAE"
    "AAQAAAACAAQAAAAAAAAAAAEAAAABAAQBAwIAAAAAAAQCAAEBAAIAAgAAAAMEAAAEAAACAQEAAAAA"
    "AAAAAAQAAQQCAAQEAgMDAAQAAAMAAAADAAAEAAEAAwAEBAQDAAACAAEAAAAABAMDAAMAAAEAAAQA"
    "AgMAAwAABAABAAIDAAQAAAICAAIAAAAAAAIEAgAAAgAEAwIAAAABAAAEAQAAAwAAAAACBAECAQAA"
    "AwAAAwQAAwQDAAAAAAACAQQDAAAAAAAEAAAAAwMBAAAAAAQAAAAAAgIAAAADBAADBAAEAAQABAAA"
    "BAAAAwQBAAAAAAACAAACAAIAAAAEAAEABAAAAgAAAAAAAAAAAAEEAAAAAwAAAQIAAAMAAQACAwQE"
    "AQABAwAAAAAAAAAAAAMBAAAABAIAAAAAAAIEAAAAAgAAAwAEAwADAAACAAEDAwQEAwAAAAAAAAAD"
    "AwACAAIDBAAABAAEAAAAAAACAgACAgICAAAAAAAAAAADAAIDAAQBAAMAAgAAAgAAAAAAAAAAAQAE"
    "AwQAAQAAAAIBAgAAAAEAAAQAAAAAAAIAAAABAQAAAwABBAADAwABAAIAAAAAAQQBAgIABAAAAAQC"
    "AAACAgMCAwQDAAAAAAACAAABAAICAAAAAgIAAAAAAQIAAAAAAAABAAAAAAAAAAAAAAIBBAQEAAQA"
    "AgQBAAEAAAAAAAAEAwAAAAAABAAAAQABAAAAAgAAAAEAAAMBAgMAAQAAAQAAAAQAAAQAAAAAAAAA"
    "AAEAAgIAAAIAAAAAAAAEAgAAAAIBAAAAAAAAAAIEAAAAAgIAAAQAAAAAAwAAAgIAAAIABAMAAQAA"
    "AAAAAAADAAAAAAAAAAADAQADBAAAAwAAAAAAAAABBAACAQAAAAABAgADAAAAAAAAAgADAAMAAAID"
    "AAIAAAAEAAAABAAAAAAAAwABAQECAwAAAAEAAAAAAAQAAAAAAAEEAAMAAAAEAAAAAAIAAwECAAAA"
    "AQAAAAABAAAAAAAABAAAAAQABAECAAIBAAECAAAAAAADAAACAgAEAAQAAAAAAAMABAAAAQEABAAA"
    "BAEAAwMEAAMAAAQABAQDBAAAAAAAAwAAAgEEAAABAAAAAAAAAAIDAgAEAQABAwACAAAEAQQEAAIA"
    "AAADAAABAgMEBAAAAAAAAgACAAAABAQAAAABAAAAAAMDAwEAAAAEAAMABAAEAwIAAAQAAQAEAAAA"
    "AgAAAAAAAAEAAAAAAAAAAwEAAAEAAgACAAAAAQADAAAAAAEAAAAAAAAABAECAAAAAAIAAAQBAgIA"
    "AwAAAAIAAAMAAAAEAAIAAAIAAQACAAAAAAAAAAAAAAMCAAADAAEBAgAAAwAAAwADAwADAAQAAAAA"
    "AAIBAwAAAQAAAAEAAAABAAAAAAAEAAEAAAQAAgQDAgEEAgMCBAAAAQIAAgAAAgIAAAABAAQAAAAA"
    "AAAAAAEAAAAAAwQAAAAAAwAEAAAAAAADAAAAAAAEAAABBAAAAAAAAwQEAAAAAgQAAAAEAgAAAAAA"
    "AAEAAAECAAAABAIEAAAAAgAAAAECAgAAAAMDAgAAAAIBAAAEAAAAAAAAAAQAAAMAAAAAAwAAAQQA"
    "AAEDAQADAAMAAAAAAAAAAAEAAAIEAAICAQAAAAIAAAAAAAEBAAEAAAAAAAACAAMDAAEAAQAAAAAA"
    "AAADAAADAAAAAAEBAwMBAwEAAAIBAAQAAAAAAAADAAAAAAEAAAMAAAABAwMAAAAAAwAABAAAAAAA"
    "AwIAAAIDBAAEAAAAAwIAAgAAAAAAAAAAAAIAAAAAAwADAAMABAMAAgQAAwAAAwAAAAAEAgADAQAE"
    "AAQAAgAEAAAAAAADAAMAAAADAgACAQQAAAAEAAEABAAAAwEABAABAgAEBAABAwMEAAAAAQAEAgEE"
    "AAMBAAAAAAAAAAAEAAAAAAEAAAABAAAAAwAAAQIAAAMAAAAAAAAAAAAAAAACAAACBAACAAAAAAIA"
    "AAICAAEAAQAAAwMAAwEBAwAEAAMDAAQCAAIEAAABBAABBAEEAAECAQMEAAAAAAACAwADBAIBAwAB"
    "AAAAAwACAgMCAAMAAAAAAwMAAAQAAAQAAQAAAAAAAAMABAQAAwAAAAEAAgABAAAABAEAAAAAAAAC"
    "AQIAAAAAAAMAAwIAAQACAQMEAwQAAAAEAAMAAQAAAAADAQABAAQAAAABAQMBAAAEAQAAAAAAAAAE"
    "AAAAAAIEAAAEAAAAAAAEAwEAAAAAAAIAAgAAAwEAAAEAAgAAAAMAAAQEAwAAAAADAQABAwAAAAAB"
    "AwADBAAEAQAAAwAABAAABAAAAAAAAAABAAAAAAMCAAAAAgEAAAQDAQAAAAMDAAAEAAIABAAAAAAA"
    "AQMEAAAAAAAAAAAAAAEEBAAEAAQDAAAAAAAAAgAAAAMAAwAAAAEAAAAAAgAAAQAAAgAEAAADBAAA"
    "AwABAAAAAwADAAICAAIAAAICAgMEAgAAAAAAAQACAAQBBAAAAQEBAAAAAAIAAAAAAgACAAIAAAAA"
    "AQAABAIDAAAAAAAAAAAAAAAEAAAAAAABAQAAAAAEAAAAAwABAwAAAAIEAAAABAEAAgMCAwACAAAC"
    "AAADAAAAAwAAAAMAAwMAAgACAAAAAAEDBAQAAwIDAAAAAAQCAgADAAADAgAAAAAAAwAAAAMBAQEA"
    "AwEAAwABAAAAAAMCAAAAAAADAAAABAQDBAAABAEAAwAAAAQEAAAAAwAAAgIBBAACAAABAAQAAAAD"
    "AAQABAICAAAEAQMAAAACBAEAAAIAAAMEAAAABAADAAAAAAIAAAMAAQAAAAABAAIAAAACAwMDAAAA"
    "AgACAAIEAAAAAAEEAAEAAAMDAAQEBAEAAAAAAAAAAAEAAgAEAAQAAAAEAAMABAABAQMAAQADAAID"
    "AAAAAAMCAgEAAwQAAgIAAAAEAAEAAAAAAAAABAAAAAAAAAQAAAAEAAAABAAAAAAAAAAAAAAAAAAA"
    "AAAEAwMAAQMAAwQAAQABAwACAAMAAAAAAAADAQAEAgAAAgIBAAQBBAAAAAAAAAQAAQAEAgAEAAIC"
    "AAIEAAIAAgAAAAADAAAABAQAAAACBAEEAwIABAACAAAAAAMABAABAAAAAAMAAAQAAAABAAMAAAAA"
    "AgACAAMAAAAAAwAAAAIAAAAAAAAAAAMEAAQEAAIAAQAAAAQDBAAAAAQABAMAAQQAAQAAAAEEAAMD"
    "AQAABAADAAAAAAABAgAAAAAABAIAAAABAAAABAABAgECAwMAAAACAgEABAABAAAAAgEBAAAEBAAC"
    "AAAAAgEAAAMAAAACAAAAAgMAAAAAAAQBAAAAAAACAQMCAAABAAADAAADAwABAAIAAAADAAADAQAA"
    "AAAABAACAAAAAAIAAAAABAMDBAQAAAAAAAQBAAQAAAAAAAAAAQAAAAEEAAMABAEAAAAEAgAAAAMA"
    "AAAAAgMCAgIAAAAAAgAAAAAAAAMAAAAAAAEAAAAAAgMBAAMAAAAABAMEAAQAAAMAAwACBAAEAAAB"
    "AAAAAAACBAQABAAEAgQAAAAEAQMDAAMAAAIEAQAEBAADAQIABAEDAAAAAgQABAADAAAAAgACBAMB"
    "AAMDAAAAAAAAAAIDAAAAAAIABAADAAAAAQAAAAAAAAAEAQAAAgABAAMDBAIBAAAABAADAAMEAwQA"
    "AAQCAAEAAwMAAAQBAAACAAABAAEAAAQCBAMBAgAAAAAAAAAABAQCAwMABAAAAAAAAAAAAAAAAQME"
    "AAAAAQAABAACAAMCAwEBAAACAgAAAgEAAAADAAAEBAAAAAAAAAABAAABAwMAAAMCAwAEAwIAAAQA"
    "BAICAAEBAAIAAAACAgIBAAAAAgQCAgAAAQQAAAAAAAAAAAMEAAADAwQABAACBAQAAwQAAQEDAQAA"
    "BAAAAAAAAwAAAAACAAMAAgMEAwEAAAAAAAEDAAAAAAIBAAQAAAMAAAMABAAEAAEEAwMAAAABBAAE"
    "AAIEAwAAAAAAAAMAAgQAAAMAAAEAAQIAAAMDBAAABAAAAAMAAAAEAAAEAAMAAAAAAAAAAAMAAAAE"
    "AAABAwAAAQAAAAEEAAAAAAIAAQAEAAAAAAADAAMAAAQDAAAAAgQCAgEAAAIBAAAAAAADBAIAAAMA"
    "AAQAAQQAAAACAAAAAAMAAgAAAQMAAAAAAQADAAIAAAAAAgAABAAAAAQEBAAEAQQAAwABAAACAAAA"
    "AAAAAAAAAAADAAAEAAABAgADAAIAAgEDAAADAAAAAAADAwQAAAMBAAAAAAAAAAAAAgABAQADAQQA"
    "BAAAAwAAAAABAAAAAAIDAAAAAwAEAAAAAQAAAAAAAwAAAAIDAAAAAwADAAQAAAEAAAECAAIABAAA"
    "BAAABAACAAMAAQAAAAIAAgIAAgAAAAQAAQACAAACAAABAAEBAAIDAAIABAAAAwEAAgMAAAAAAAMA"
    "BAACBAAAAAAABAABBAAEAAAAAQQAAQAAAAAEAgAAAAAAAwADAAAAAAAAAAMAAAAAAAEAAAAABAEA"
    "AAAEAgIAAAIAAAAAAAAAAAAAAAEEAAADAAAAAAEAAwAAAAMEAgAAAAAAAAAAAAIEAAEAAQAABAAA"
    "BAEAAAQAAwAAAwABAAIDAwQEAAAAAwQAAAQABAMAAAECAgACAAIDAAAAAQIEAAQABAQDAAAAAAAA"
    "AAAAAAAAAwABAwAAAQADAwIAAAAAAQABAAAAAAEABAQBAwABAAADAgAEAAIAAAMABAEAAAEAAQAA"
    "BAMAAwQCAwMAAQMCAwQAAwAAAAEABAAAAAEAAgEAAAAAAAAAAAAAAAAAAgAEAQAAAAEAAAAEAwAA"
    "AQIABAMEAAABAAMAAgEEAAIAAAEEAAABAAABAQAAAAAAAgIAAAAAAAADAgABBAMEAgACBAACBAQA"
    "AgADAAACAgQAAwADAwAEBAQAAAEBAAAABAECAAAAAAAABAACAAAEBAAAAAADAAAEAAMAAAIBAAAA"
    "AAQAAQAABAAAAAACAAEDAwAEBAAAAAAAAAACAQAAAAAEAAIAAAADAAAAAAIAAwAAAAEEBAAAAgAD"
    "AAAAAgEAAAQAAAEAAAAAAAIEAAMAAwQABAACAAEBAAEAAAEABAAAAAICBAQAAQAAAgIEAAAAAAAA"
    "AAAAAAAABAIBAAAAAgIAAAACAQAAAAABAAAAAAQEAgAEAAABAAAAAAAAAAEAAAMCAwAEBAMDBAAA"
    "AAABAAABAAEBAAABAwAAAAABAAABAwMAAAABAAMEAAAAAgAAAAQAAAACAAMAAAAAAAAAAAQAAAQD"
    "AAAABAABAAIAAAIAAAAAAAICAwACAwABAAAAAAQAAwADAgAAAAAAAgEABAIAAAAAAAABBAAAAAIC"
    "AAQAAAQAAAEAAwMDAAAAAQAEBAAAAAEAAAEBAAAAAgAAAwIABAADAAAEAgAAAAAABAAAAAAAAAAC"
    "AAQAAgAEAwAAAAAEAAMEBAEAAQACAAAEAAAABAAAAAAAAAAEAQQAAAQEAAQAAgAAAQEAAQAAAAQE"
    "AAABAAAAAAQABAAEAQAABAACAwACBAQEAAAAAQEAAQABAAAAAAAAAAAAAQAAAQAAAAAEAAACAAAA"
    "BAACAAEAAAAAAAMAAAIAAAMEAQAAAAIBAAIBAAAABAECAAAAAAAAAAABAAMBAAAAAwQAAgAAAwAA"
    "AwAEAQQAAwAAAQQAAwQAAAABAAABAAAEAAQAAAACAAABAAAAAAAAAQIAAAABAAAAAAICAAACAAIA"
    "AAADAgMCAAABAAAAAwACAAMABAAAAAAAAAAAAAAAAAIAAAAAAAQBAAAAAAECAQMBAAAAAAACAAAD"
    "AAAAAAQCAAQBAAACAAAAAAMAAwIAAgMAAAABAwMDBAAABAAEAAAAAAEBAAQCAQAEAAQABAIAAAID"
    "AAEAAQAAAAACAAQAAAABAAADAQECAAAAAAQAAAMABAACAAAAAAQAAAAAAAAAAQEDAAABAwQDAwIA"
    "BAAAAQADAAAAAgAEAwAABAABAQAABAABAAQAAgAAAAAAAAQAAAMBAAACBAAEAAEEAAAABAAABAAA"
    "AAAABAMDAAEBAAAAAAAEAgMAAAAEAgADAAACAgAAAAMAAAQBAQAAAQAEAgAAAAMDAAAAAAABBAAA"
    "AAAAAwQBAAIAAAABAAIAAAIABAMAAAAEAwMAAAABAAAAAwECBAAABAAAAAACAAAAAAAAAAAEAQIB"
    "AAAABAMAAAQCAwEBAgAAAAQAAQAAAAABAAAAAAIAAwACAwECAQAAAgMCAwAEAAAEAQQAAAAAAwAA"
    "AAMAAAMAAAAABAAAAAAAAAMAAAMEAAAAAAAEAAAAAAAAAAQAAwECAAQAAAAAAgAAAAAAAAAAAAAA"
    "AAAEAAADAwAAAAMCAAIAAAAAAwAAAgADAAACAAADAAAAAAMBAAEBAAECAAADAAAEAQMDBAACAAAC"
    "AAABAAACAAQAAAAAAQAAAAAAAQABAwQAAAQCAAAAAwMAAQADAAMAAAMAAAIAAAAAAAAAAAEEAAAA"
    "AAMAAAMEAAACAAAAAAMAAwIAAQMAAgIAAAIAAQAAAAAABAMAAAAAAgEAAAABAQEBAAQAAgQDAAAA"
    "BAMAAAEAAAAAAgIAAwMAAAAABAIAAAADAAECAgIAAAEBAAMBAAQAAgAAAAIAAAIAAAAAAAQEAAAD"
    "AQEEAQIDAAACAAACAAIEAAECAAAAAgMCAwACAAABAwAAAwAAAAAABAAEAAQDAAAAAAABAQEBAAAE"
    "AAAAAwAAAgAAAAADAAECAQMAAAABAAACAAAAAAAAAwMAAAIAAAIAAAEBAAIEAAAEAAAAAAAAAAMA"
    "AQQAAAMEAAMAAwMAAQAAAAAAAAMEAAQCAAIDAAMDBAQAAAAEAAEAAAMCAQACAgAAAAEDAAQAAwAA"
    "AAAAAQQAAAICBAMAAAEAAAAAAAQDAAAAAQAAAQADAAADAAAAAAAAAQAABAAAAAAAAQADAgICAQIA"
    "AAIBAAEAAwAAAAAAAAADAwAAAAAABAIAAAAAAAAEAAMABAAAAAAAAAQAAwQABAAAAAAAAAAAAwED"
    "AAMAAAAAAAAABAMAAAAAAwEAAgABAAAAAQAAAAACAAAAAAAEAQABAAABAQAAAQAAAAMAAgABAAMA"
    "AAAABAAEAQAAAAMABAAAAAEAAQAAAwQDAAACAAQEAAACAAAEBAAAAAMBAAABAAACAAAAAAQAAAAB"
    "AAADAQIBAAADAAEAAQAAAgMBAAADAAIDAAQAAAAAAQEBAQAAAgMAAAACAAAEAwABAAAAAAAEAAAD"
    "AAEEAwEAAQAAAQACAAEAAAMAAQMAAgAAAAIAAAQAAAAAAAIDAAAAAAA="
)

_RANK_CACHE = None


def _get_rank() -> np.ndarray:
    """rank[i]: which of the 5 nearest negatives the reference picks per row.

    Must reproduce the reference's jax.random draws bit-exactly; compute on
    the CPU jax backend when available, else use the embedded constant
    (generated the same way).
    """
    global _RANK_CACHE
    if _RANK_CACHE is not None:
        return _RANK_CACHE
    try:
        import jax
        import jax.numpy as jnp

        cpu = jax.devices("cpu")[0]
        with jax.default_device(cpu):
            k1, k2 = jax.random.split(jax.random.key(1))
            coin = jax.random.uniform(k1, (B,)) < 0.5
            rank = jnp.where(coin, 0, jax.random.randint(k2, (B,), 0, HARD_RANK))
            r = np.asarray(jax.device_get(rank)).astype(np.uint8)
    except Exception:
        r = np.frombuffer(base64.b64decode(_RANK_B64), dtype=np.uint8)
    assert r.shape == (B,)
    _RANK_CACHE = r
    return r


_NC_CACHE = None


def _build_nc():
    import os as _os

    NBUF = int(_os.environ.get("K_NBUF", "8"))

    import concourse.mybir as mybir
    import concourse.tile as tile
    from concourse import bacc

    F32 = mybir.dt.float32
    BF16 = mybir.dt.bfloat16
    FP8 = mybir.dt.float8e4
    PM = mybir.MatmulPerfMode

    nc = bacc.Bacc()
    # at8[p, t*M + i] = 32*ahat.T[t*128+p, i]   (core's anchor slab)
    at8 = nc.dram_tensor("at8", [128, 2 * M], FP8, kind="ExternalInput").ap()
    # pt8[p, k*2*GW + t*GW + j] = 16*phat.T[t*128+p, k*GW+j]: column chunks of
    # GW cols, both k-tiles packed per chunk, so each chunk DMAs separately
    pt8 = nc.dram_tensor("pt8", [128, 2 * B], FP8, kind="ExternalInput").ap()
    # per (row, rb): 16 direct candidates (f32, two 1024-col PSUM Max8s)
    # plus 768 partially-reduced tree maxima (bf16) merged host-side
    cd = nc.dram_tensor("cd", [128, RB * 16], F32, kind="ExternalOutput").ap()
    ct = nc.dram_tensor("ct", [128, RB * 768], BF16, kind="ExternalOutput").ap()

    with tile.TileContext(nc) as tc:
        with (
            tc.tile_pool(name="ops", bufs=1) as opsp,
            tc.tile_pool(name="evac", bufs=NBUF) as evacp,
            tc.tile_pool(name="tree", bufs=NBUF) as treep,
            tc.tile_pool(name="out", bufs=2) as outp,
            tc.tile_pool(name="psA", bufs=2, space="PSUM") as psA,
            tc.tile_pool(name="psD", bufs=1, space="PSUM") as psD,
        ):
            # at8 packed [p, rb*256 + t*128 + m]; rb0's slice lands first so
            # the pipeline starts as soon as column chunk 1 arrives
            # PE warmup fodder: ramp the p-state during the DMA head
            warm = opsp.tile([128, 128], FP8, tag="warm")
            nc.gpsimd.memset(warm, 0.0)
            # rb0's anchor slice is its own tile so its matmuls don't wait
            # for the full anchor DMA (deps are tile-granular)
            a0 = opsp.tile([128, 256], FP8, tag="a0")
            nc.sync.dma_start(a0, at8[:, :256])
            a_sb = opsp.tile([128, 2 * M - 256], FP8)
            # columns 0..4095 land as four separate 1024-col tiles (DMA deps
            # are tile-granular) in consumption order; cols 4096+ as two
            # 2048-col tiles; the anchor tail (blocks 1-7) trails
            psub = []
            for s in range(4):
                t0 = opsp.tile([128, 2048], FP8, tag=f"ps{s}", name=f"psub{s}")
                nc.sync.dma_start(t0, pt8[:, s * 2048 : (s + 1) * 2048])
                psub.append(t0.rearrange("p (t n) -> p t n", t=2))
            pcol = [None, None, None, None]
            for k in (2, 3):
                pc = opsp.tile([128, 2 * GW], FP8, tag=f"pc{k}", name=f"pc{k}")
                nc.sync.dma_start(pc, pt8[:, k * 2 * GW : (k + 1) * 2 * GW])
                pcol[k] = pc.rearrange("p (t n) -> p t n", t=2)
            nc.sync.dma_start(a_sb, at8[:, 256:])
            a0v = a0.rearrange("p (t m) -> p t m", t=2)
            lhsT_all = a_sb.rearrange("p (r t m) -> p r t m", r=RB - 1, t=2)

            def rhs_for(col):
                """SBUF view + local offset for a 512-col matmul at `col`."""
                if col < 4096:
                    return psub[col // 1024], col % 1024
                return pcol[col // GW], col % GW

            AW = 1536  # ACT granule width (3 PSUM banks)
            DW = 1024  # DVE granule width (2 PSUM banks)
            NA = 4  # ACT granules per row block (even -> no slot bubble)
            ND = 2

            for rb in range(RB):
                lhsT = a0v if rb == 0 else lhsT_all[:, rb - 1]

                def fill(pool, w, c0, tag, warmup=False, lhsT=lhsT):
                    ps = pool.tile([128, w], F32, tag=tag)
                    if warmup:
                        # ramp the PE p-state while the input DMAs land (the
                        # real matmuls below overwrite with start=True)
                        wv = warm.rearrange("p (t n) -> p t n", t=2)
                        for _ in range(2):
                            nc.tensor.matmul(
                                ps[:64, :64], wv, wv,
                                start=True, stop=True, perf_mode=PM.DoubleRow,
                            )
                    for q in range(w // MMW):
                        view, off = rhs_for(c0 + q * MMW)
                        nc.tensor.matmul(
                            ps[:, q * MMW : (q + 1) * MMW],
                            lhsT,
                            view[:, :, off : off + MMW],
                            start=True, stop=True, perf_mode=PM.DoubleRow,
                        )
                    return ps

                hs = []
                for g in range(NA):
                    ps = fill(psA, AW, g * AW, "psA", warmup=(rb == 0 and g == 0))
                    ev = evacp.tile([128, AW], BF16, tag=f"e{g}")
                    nc.scalar.copy(ev, ps)
                    h = treep.tile([128, AW // 2], BF16, tag=f"h{g}")
                    nc.vector.tensor_max(h, ev[:, : AW // 2], ev[:, AW // 2 :])
                    hs.append(h)
                for d in range(ND):
                    ps = fill(psD, DW, NA * AW + d * DW, "psD")
                    c8 = outp.tile([128, 8], F32, tag=f"c8{d}")
                    nc.vector.max(out=c8, in_=ps)
                    lo = rb * 16 + d * 8
                    nc.sync.dma_start(cd[:, lo : lo + 8], c8)
                # merge chain; host takes the top of the shipped 768
                m1 = treep.tile([128, AW // 2], BF16, tag="m1")
                nc.vector.tensor_max(m1, hs[0], hs[1])
                m2 = treep.tile([128, AW // 2], BF16, tag="m2")
                nc.vector.tensor_max(m2, m1, hs[2])
                m3 = treep.tile([128, AW // 2], BF16, tag="m3")
                nc.vector.tensor_max(m3, m2, hs[3])
                nc.sync.dma_start(ct[:, rb * 768 : (rb + 1) * 768], m3)

    nc.compile()
    return nc


def _get_nc():
    global _NC_CACHE
    if _NC_CACHE is None:
        _NC_CACHE = _build_nc()
    return _NC_CACHE


def _prep(x: np.ndarray):
    """Host prep: normalize, scale, transpose, interleave, fp8-quantize."""
    x = np.ascontiguousarray(np.asarray(x, dtype=np.float32))
    assert x.shape == (B, 2, D)
    x0 = x[:, 0, :]
    x1 = x[:, 1, :]
    na = np.sqrt(np.sum(x0 * x0, axis=1, keepdims=True))
    np_ = np.sqrt(np.sum(x1 * x1, axis=1, keepdims=True))
    ahat = x0 / np.maximum(na, 1e-12)
    phat = x1 / np.maximum(np_, 1e-12)

    a8 = (ASCALE * ahat).astype(ml_dtypes.float8_e4m3)
    p8 = (PSCALE * phat).astype(ml_dtypes.float8_e4m3)
    # the device sees the fp8-rounded values; use them for the exact diag
    a8f = a8.astype(np.float32)
    p8f = p8.astype(np.float32)
    sii_dev = np.einsum("ij,ij->i", a8f, p8f)  # approx of diag psum value

    aT = np.ascontiguousarray(a8.T)  # [D, B]
    pT = np.ascontiguousarray(p8.T)  # [D, B]

    pt8 = np.empty((128, 2 * B), dtype=ml_dtypes.float8_e4m3)
    # cols 0..4095 as four 1024-col sub-chunks, cols 4096+ as 2048-col chunks
    for s in range(4):
        for t in range(2):
            pt8[:, s * 2048 + t * 1024 : s * 2048 + (t + 1) * 1024] = pT[
                t * 128 : (t + 1) * 128, s * 1024 : (s + 1) * 1024
            ]
    for k in (2, 3):
        for t in range(2):
            pt8[:, k * 2 * GW + t * GW : k * 2 * GW + (t + 1) * GW] = pT[
                t * 128 : (t + 1) * 128, k * GW : (k + 1) * GW
            ]

    in_maps = []
    for c in range(NCORES):
        # at8[p, rb*256 + t*128 + m] = aT[t*128+p, c*M + rb*128 + m]
        at8 = np.empty((128, 2 * M), dtype=ml_dtypes.float8_e4m3)
        for rb in range(RB):
            for t in range(2):
                lo = rb * 256 + t * 128
                at8[:, lo : lo + 128] = aT[
                    t * 128 : (t + 1) * 128,
                    c * M + rb * 128 : c * M + (rb + 1) * 128,
                ]
        in_maps.append({"at8": np.ascontiguousarray(at8), "pt8": pt8})
    return in_maps, ahat, phat, sii_dev


def _epilogue(cands: np.ndarray, ahat, phat, sii_dev) -> np.float32:
    """cands: [B, 16] raw psum-scale candidate values, unsorted."""
    rank = _get_rank()

    order = np.argsort(-cands, axis=1)
    csort = np.take_along_axis(cands, order, axis=1)  # desc [B, 24]

    # drop the self-match: closest candidate to the (device-precision) diag
    # value, if within the fp8 noise band
    TOL = 8.0  # psum units; fp8 dot noise sigma ~1.7, bf16 evac ~0.5
    diff = np.abs(csort - sii_dev[:, None])
    kstar = np.argmin(diff, axis=1)
    hit = diff[np.arange(B), kstar] < TOL
    # shift left past the dropped slot where hit
    idx = np.arange(8)[None, :] + (
        hit[:, None] & (np.arange(8)[None, :] >= kstar[:, None])
    )
    top = np.take_along_axis(csort, idx, axis=1)  # [B, 8] diag-free

    s_sel = top[np.arange(B), rank] / SS  # = <ahat_i, phat_neg>
    ra = np.sum(ahat, axis=1)
    pos_d2 = np.sum(np.square(ahat - phat + EPS), axis=1)
    neg_d2 = 2.0 - 2.0 * s_sel + 2.0 * EPS * ra + D * EPS * EPS
    return np.float32(np.mean(np.maximum(pos_d2 - neg_d2, 0.0)))


def kernel(x: np.ndarray, _want_timing: bool = False):
    """x: [8192, 2, 256] float32 -> scalar float32 loss (0-d ndarray)."""
    from concourse.bass_utils import run_bass_kernel_spmd

    in_maps, ahat, phat, sii_dev = _prep(x)
    nc = _get_nc()
    res = run_bass_kernel_spmd(nc, in_maps, list(range(NCORES)))

    cands = np.empty((B, 28), dtype=np.float32)
    for c in range(NCORES):
        cdv = res.results[c]["cd"]  # [128, RB*16] f32
        ctv = res.results[c]["ct"]  # [128, RB*768] bf16
        for rb in range(RB):
            rows = slice(c * M + rb * 128, c * M + (rb + 1) * 128)
            cands[rows, 0:16] = cdv[:, rb * 16 : (rb + 1) * 16]
            tree = ctv[:, rb * 768 : (rb + 1) * 768].astype(np.float32)
            # top-12 of the 768 partially-reduced tree maxima per row
            cands[rows, 16:28] = np.partition(tree, 768 - 12, axis=1)[:, -12:]

    out = _epilogue(cands, ahat, phat, sii_dev)
    if _want_timing:
        return np.asarray(out), res, cands
    return np.asarray(out)


if __name__ == "__main__":
    rng = np.random.default_rng(0)
    x = rng.standard_normal((B, 2, D)).astype(np.float32)
    print(kernel(x))
